# revision 10
# baseline (speedup 1.0000x reference)
"""Trainium2 Bass kernel for 16-head MHA (B=2, T=2048, E=1024), SPMD on 8 cores.

Sharding: data-parallel over batch (2) x tensor-parallel over heads (4 groups
of 4 heads). Each core computes, for its (batch b, head-group g):
  qk^T projection (feature-major), v projection (token-major),
  shifted-softmax attention via an augmented-row matmul trick, and a partial
  output projection over its 256 embedding columns. The host sums the 4
  partial projections per batch.

Softmax shift: the exact per-query max over all keys is computed on-device
(q-stationary matmul in [i, j] orientation + free-dim DVE reduces) and folded
into the main QK^T matmul as a rank-1 augmented row, so scores arrive in PSUM
already shifted: S'[j,i] = 8*q_i.k_j - M_i. exp() runs on ACT straight out of
PSUM. The softmax denominator comes for free from a ones-column appended to V.
"""

import sys

sys.path.insert(0, "/opt/trn_rl_repo")

import numpy as np

import concourse.bass as bass
import concourse.mybir as mybir
import concourse.tile as tile_mod
from concourse.masks import make_identity

F32 = mybir.dt.float32
F16 = mybir.dt.float16

B, T, E = 2, 2048, 1024
H_TOTAL, D = 16, 64
N_CORES = 8
GROUPS = 4          # head-group (tensor) parallelism
HPG = H_TOTAL // GROUPS  # 4 heads per group
DV = HPG * D        # 256: v width / out-proj contraction per core
FQK = 2 * DV        # 512: q+k feature rows per core
SCALE = float(np.sqrt(D))  # reference MULTIPLIES scores by sqrt(d)

NE = E // 128       # 8 e-chunks
NT_TILE = T // 128  # 16 token tiles
NT_CHUNK = T // 512  # 4 token chunks
N_SAMPLE_TILES = 2  # 256-key sample for the softmax shift


# ---------------------------------------------------------------------------
# Workaround: this walrus build only accepts ONE sem wait per instruction.
# After Tile scheduling, split every multi-wait instruction: the overflow
# waits move onto same-engine NoOps inserted immediately before it.
def _split_multi_waits(nc):
    for f in nc.m.functions:
        for bb in f.blocks:
            out = []
            for inst in bb.instructions:
                si = getattr(inst, "sync_info", None)
                if si is not None and si.on_wait and len(si.on_wait) > 1:
                    extras = list(si.on_wait[:-1])
                    si.on_wait = list(si.on_wait[-1:])
                    for w in extras:
                        nop = mybir.InstNoOp(
                            name=f"I-{nc.next_id()}", ins=[], outs=[]
                        )
                        nop.engine = inst.engine
                        nop.sync_info = mybir.SyncInfo(on_wait=[w], on_update=[])
                        out.append(nop)
                out.append(inst)
            bb.instructions[:] = out


# ---------------------------------------------------------------------------
# Device program (identical on every core; per-core data differs)
def _emit_body(nc, tc, dram, ctx_pools, dbg=None):
    xT_d, wqkT_d, wvT_d, woutT_d, y_d = dram
    persist = ctx_pools["persist"]

    # persistent SBUF
    qk_sb = [persist.tile([128, T], F16, tag=f"qk{i}", name=f"qk{i}") for i in range(FQK // 128)]
    # v as one [128, 16*256] tile: [t-tile partition, (jt, dv)] layout
    v_sb = persist.tile([128, NT_TILE * DV], F16, tag="v", name="v")
    oall_sb = [persist.tile([128, T], F16, tag=f"oall{i}", name=f"oall{i}") for i in range(DV // 128)]
    wout_sb = [persist.tile([128, E], F16, tag=f"wout{i}", name=f"wout{i}") for i in range(DV // 128)]
    ones_t = persist.tile([D + 1, D], F16, tag="ones_t", name="ones_t")
    nc.vector.memset(ones_t, 1.0)
    identity = persist.tile([128, 128], F32, tag="identity", name="identity")
    make_identity(nc, identity)
    for i in range(DV // 128):
        nc.sync.dma_start(out=wout_sb[i], in_=woutT_d[i * 128:(i + 1) * 128, :])

    # ---- Phase 1: projections ----------------------------------------
    # ff order (0,2,1,3): heads 0/1 need qk tiles 0 and 2 first. qk copies
    # go via ACT (idle in phase 1); v copies stay on DVE.
    with (
        tc.tile_pool(name="ph1", bufs=1) as ph1,
        tc.tile_pool(name="pj", bufs=4, space="PSUM") as pj,
        tc.tile_pool(name="pv", bufs=2, space="PSUM") as pv,
    ):
        xt_sb = [ph1.tile([128, T], F16, tag=f"xt{i}", name=f"xt{i}") for i in range(NE)]
        wqk_sb = [ph1.tile([128, FQK], F16, tag=f"wqk{i}", name=f"wqk{i}") for i in range(NE)]
        wv_sb = [ph1.tile([128, DV], F16, tag=f"wv{i}", name=f"wv{i}") for i in range(NE)]
        for i in range(NE):
            nc.sync.dma_start(out=xt_sb[i], in_=xT_d[i * 128:(i + 1) * 128, :])
            nc.sync.dma_start(out=wqk_sb[i], in_=wqkT_d[i * 128:(i + 1) * 128, :])
            nc.sync.dma_start(out=wv_sb[i], in_=wvT_d[i * 128:(i + 1) * 128, :])

        # qk^T [f', t] = W'[f', e] @ x^T[e, t], feature-major
        for ff in (0, 2, 1, 3):
            ps = [pj.tile([128, 512], F32, tag="pj", name="pj") for _ in range(NT_CHUNK)]
            for ne in range(NE):
                lhsT = wqk_sb[ne][:, ff * 128:(ff + 1) * 128]
                for tt in range(NT_CHUNK):
                    nc.tensor.matmul(
                        ps[tt],
                        lhsT,
                        xt_sb[ne][:, tt * 512:(tt + 1) * 512],
                        start=(ne == 0),
                        stop=(ne == NE - 1),
                    )
            for tt in range(NT_CHUNK):
                nc.scalar.activation(
                    out=qk_sb[ff][:, tt * 512:(tt + 1) * 512], in_=ps[tt],
                    func=mybir.ActivationFunctionType.Copy,
                )

        # v [t, dv] token-major
        for tj in range(NT_TILE):
            psv = pv.tile([128, DV], F32, tag="pv", name="pv")
            for ne in range(NE):
                nc.tensor.matmul(
                    psv,
                    xt_sb[ne][:, tj * 128:(tj + 1) * 128],
                    wv_sb[ne],
                    start=(ne == 0),
                    stop=(ne == NE - 1),
                )
            nc.scalar.activation(
                out=v_sb[:, tj * DV:(tj + 1) * DV], in_=psv,
                func=mybir.ActivationFunctionType.Copy,
            )

    # ---- Phase 2: attention -------------------------------------------
    # Partition-alignment rules (DVE/ACT lanes are hardwired per partition):
    # odd heads' q/k rows live at SBUF partitions 64..127 of qk_sb, so the
    # copy into base-0 aug tiles goes via DMA (address-based, can shift
    # partitions). The per-head V-stationary tile [128, jt*128] puts the
    # ones column and O rows at parity-dependent positions so every later
    # DVE op sees matching base partitions.
    with (
        tc.tile_pool(name="aug", bufs=1) as augp,
        tc.tile_pool(name="vaug", bufs=1) as vaugp,
    ):
        k_augs, q_augs, vaugs = [], [], []
        v_r = v_sb.rearrange("p (j c) -> p j c", c=DV)
        for h in range(HPG):
            odd = h % 2 == 1
            q_tile, k_tile = h // 2, 2 + h // 2
            k_aug = augp.tile([D + 1, T], F16, tag=f"kaug{h}", name=f"kaug{h}")
            q_aug = augp.tile([D + 1, T], F16, tag=f"qaug{h}", name=f"qaug{h}")
            if odd:
                nc.sync.dma_start(out=k_aug[0:D, :], in_=qk_sb[k_tile][D:2 * D, :])
                nc.sync.dma_start(out=q_aug[0:D, :], in_=qk_sb[q_tile][D:2 * D, :])
            else:
                nc.vector.tensor_copy(out=k_aug[0:D, :], in_=qk_sb[k_tile][0:D, :])
                nc.vector.tensor_copy(out=q_aug[0:D, :], in_=qk_sb[q_tile][0:D, :])
            nc.vector.memset(k_aug[D:D + 1, :], 1.0)
            k_augs.append(k_aug)
            q_augs.append(q_aug)
        for h in range(HPG):
            odd = h % 2 == 1
            # V-stationary [128, 16*128]: per j-tile 128 columns holding
            # (even) [v(64) | ones | 0...]   -> O rows 0..63, denom row 64
            # (odd)  [ones | 0... | v(64)]   -> denom row 0, O rows 64..127
            vaug = vaugp.tile([128, NT_TILE * 128], F16, tag=f"vaug{h}", name=f"vaug{h}")
            va_r = vaug.rearrange("p (j c) -> p j c", c=128)
            eng = nc.gpsimd if odd else nc.vector
            if odd:
                eng.memset(va_r[:, :, 0:1], 1.0)
                eng.memset(va_r[:, :, 1:D], 0.0)
                eng.tensor_copy(out=va_r[:, :, D:2 * D], in_=v_r[:, :, h * D:(h + 1) * D])
            else:
                eng.tensor_copy(out=va_r[:, :, 0:D], in_=v_r[:, :, h * D:(h + 1) * D])
                eng.memset(va_r[:, :, D:D + 1], 1.0)
                eng.memset(va_r[:, :, D + 1:128], 0.0)
            vaugs.append(vaug)

        # pass 0: exact per-query max over ALL keys, all heads. Scores are
        # computed [i-part, j-free] (q-stationary) so the row max is a
        # free-dim reduce per 512-key chunk (split DVE/Pool), max-combined
        # across chunks; a tiny PE transpose + 4-descriptor DMA then lays
        # -M along the free dim of q_aug row D.
        with (
            tc.tile_pool(name="mx", bufs=1) as mxp,
            tc.tile_pool(name="px", bufs=4, space="PSUM") as pxp,
            tc.tile_pool(name="mx4", bufs=2, space="PSUM") as mx4p,
        ):
            for h in range(HPG):
                q_aug, k_aug = q_augs[h], k_augs[h]
                for ic in range(NT_CHUNK):
                    ics = slice(ic * 512, (ic + 1) * 512)
                    mxt = mxp.tile([128, NT_CHUNK], F32, tag="mxt", name="mxt", bufs=2)
                    mxt2 = mxp.tile([128, NT_CHUNK], F32, tag="mxt2", name="mxt2", bufs=2)
                    for c in range(NT_CHUNK):
                        it = ic * NT_CHUNK + c
                        for jc in range(NT_CHUNK):
                            s_ps = pxp.tile([128, 512], F32, tag="px", name="px")
                            nc.tensor.matmul(
                                s_ps,
                                q_aug[0:D, it * 128:(it + 1) * 128],
                                k_aug[0:D, jc * 512:(jc + 1) * 512],
                                start=True,
                                stop=True,
                            )
                            if jc == 0:
                                nc.vector.reduce_max(
                                    out=mxt[:, c:c + 1], in_=s_ps,
                                    axis=mybir.AxisListType.X,
                                )
                            else:
                                nc.vector.reduce_max(
                                    out=mxt2[:, c:c + 1], in_=s_ps,
                                    axis=mybir.AxisListType.X,
                                )
                                nc.vector.tensor_max(
                                    mxt[:, c:c + 1], mxt[:, c:c + 1],
                                    mxt2[:, c:c + 1],
                                )
                    mx4 = mx4p.tile([NT_CHUNK, 128], F32, tag="mx4", name="mx4")
                    nc.tensor.transpose(mx4, mxt, identity)
                    mrow = mxp.tile([NT_CHUNK, 128], F16, tag="mrow", name="mrow", bufs=2)
                    nc.vector.tensor_scalar_mul(mrow, mx4, -1.0)
                    nc.sync.dma_start(
                        out=q_aug[D:D + 1, ics].rearrange(
                            "p (c f) -> p c f", c=NT_CHUNK
                        ),
                        in_=mrow,
                    )

        # main: S' = qk - M (augmented matmul), exp on ACT, PV accumulate,
        # normalize; out-projection for the finished token chunk follows
        # immediately (interleaved phase 3).
        with (
            tc.tile_pool(name="pt", bufs=4) as ptp,
            tc.tile_pool(name="rr", bufs=2) as rrp,
            tc.tile_pool(name="ot", bufs=2) as otp,
            tc.tile_pool(name="ysb", bufs=3) as ysbp,
            tc.tile_pool(name="ps", bufs=2, space="PSUM") as psp,
            tc.tile_pool(name="po", bufs=2, space="PSUM") as pop,
            tc.tile_pool(name="pr", bufs=2, space="PSUM") as prp,
            tc.tile_pool(name="py", bufs=2, space="PSUM") as pyp,
        ):
            def normalize(ic, h, po):
                # r = 1/denominator (partition `den`), replicate across the
                # 64 O-row partitions via ones outer-product matmul (PE),
                # multiply into oall. Emitted one (ic,h) step late so the
                # slow single-partition reciprocal hides under the next
                # head's matmul block instead of stalling the PE.
                ics = slice(ic * 512, (ic + 1) * 512)
                odd = h % 2 == 1
                o_base = (h % 2) * D
                den = D if not odd else 0
                r = rrp.tile([D + 1, 512], F16, tag="r", name="r")
                nc.vector.reciprocal(
                    out=r[den:den + 1, :], in_=po[den:den + 1, :]
                )
                pr = prp.tile([128, 512], F32, tag="pr", name="pr")
                nc.tensor.matmul(
                    pr[o_base:o_base + D, :],
                    ones_t[den:den + 1, 0:D],
                    r[den:den + 1, :],
                    start=True,
                    stop=True,
                )
                ot = otp.tile([128, 512], F32, tag="ot", name="ot")
                nc.scalar.activation(
                    out=ot[o_base:o_base + D, :], in_=po[o_base:o_base + D, :],
                    func=mybir.ActivationFunctionType.Copy,
                )
                nc.vector.tensor_mul(
                    oall_sb[h // 2][o_base:o_base + D, ics],
                    ot[o_base:o_base + D, :],
                    pr[o_base:o_base + D, :],
                )

            def phase3(ic):
                # out-projection for a finished token chunk
                for tt in range(ic * NT_CHUNK, (ic + 1) * NT_CHUNK):
                    pys = [pyp.tile([128, 512], F32, tag="py", name="py") for _ in range(2)]
                    for es in range(DV // 128):
                        lhsT = oall_sb[es][:, tt * 128:(tt + 1) * 128]
                        for oc in range(2):
                            nc.tensor.matmul(
                                pys[oc],
                                lhsT,
                                wout_sb[es][:, oc * 512:(oc + 1) * 512],
                                start=(es == 0),
                                stop=(es == DV // 128 - 1),
                            )
                    yt = ysbp.tile([128, E], F32, tag="y", name="y")
                    for oc in range(2):
                        nc.vector.tensor_copy(out=yt[:, oc * 512:(oc + 1) * 512], in_=pys[oc])
                    nc.sync.dma_start(out=y_d[tt * 128:(tt + 1) * 128, :], in_=yt)

            pending = None  # (ic, h, po) awaiting normalize
            for ic in range(NT_CHUNK):
                ics = slice(ic * 512, (ic + 1) * 512)
                for h in range(HPG):
                    q_aug, k_aug, vaug = q_augs[h], k_augs[h], vaugs[h]
                    po = pop.tile([128, 512], F32, tag="po", name="po")
                    for jt in range(NT_TILE):
                        ps = psp.tile([128, 512], F32, tag="ps", name="ps")
                        nc.tensor.matmul(
                            ps,
                            k_aug[:, jt * 128:(jt + 1) * 128],
                            q_aug[:, ics],
                            start=True,
                            stop=True,
                        )
                        pT = ptp.tile([128, 512], F16, tag="pt", name="pt")
                        nc.scalar.activation(
                            out=pT, in_=ps, func=mybir.ActivationFunctionType.Exp
                        )
                        nc.tensor.matmul(
                            po,
                            vaug[:, jt * 128:(jt + 1) * 128],
                            pT,
                            start=(jt == 0),
                            stop=(jt == NT_TILE - 1),
                        )
                    if pending is not None:
                        pic, ph, ppo = pending
                        normalize(pic, ph, ppo)
                        if ph == HPG - 1:
                            phase3(pic)
                    pending = (ic, h, po)
            pic, ph, ppo = pending
            normalize(pic, ph, ppo)
            phase3(pic)


def _build_nc(reps=1, debug=False, split_waits=True):
    nc = bass.Bass()
    xT_d = nc.declare_dram_parameter("xT", [E, T], F16, isOutput=False)
    wqkT_d = nc.declare_dram_parameter("wqkT", [E, FQK], F16, isOutput=False)
    wvT_d = nc.declare_dram_parameter("wvT", [E, DV], F16, isOutput=False)
    woutT_d = nc.declare_dram_parameter("woutT", [DV, E], F16, isOutput=False)
    y_d = nc.declare_dram_parameter("y", [T, E], F32, isOutput=True)
    dram = (xT_d, wqkT_d, wvT_d, woutT_d, y_d)
    dbg = None
    if debug:
        shapes = {
            "qk0": [128, T],
            "qk2": [128, T],
            "v0": [128, NT_TILE * DV],
            "qaug0": [D + 1, T],
            "kaug0": [D + 1, T],
            "vaug0": [128, NT_TILE * 128],
            "oall0": [128, T],
            "den0": [NT_CHUNK, 512],
            "sp0": [128, T],
        }
        keys = debug if isinstance(debug, (list, tuple)) else list(shapes)
        dbg = {
            k: nc.declare_dram_parameter(k, shapes[k], F32, isOutput=True)
            for k in keys
        }
    with tile_mod.TileContext(nc) as tc, nc.allow_low_precision(
        reason="fp16 kernel: scores/softmax accumulate in fp32 PSUM; fp16 "
        "elsewhere is validated to rel-err ~5e-3 vs the fp64 reference"
    ):
        for _ in range(reps):
            with tc.tile_pool(name="persist", bufs=1) as persist:
                _emit_body(nc, tc, dram, {"persist": persist}, dbg=dbg)
    if split_waits:
        _split_multi_waits(nc)
    return nc


# ---------------------------------------------------------------------------
# Execution: cached jitted shard_map over 8 cores (axon/PJRT path)
_RUNNERS = {}


class _Runner:
    def __init__(self, reps=1, debug=False):
        import jax
        from jax.sharding import Mesh, PartitionSpec
        from jax.experimental.shard_map import shard_map
        from concourse import bass2jax

        bass2jax.install_neuronx_cc_hook()
        nc = self._nc = _build_nc(reps, debug=debug)

        partition_name = (
            nc.partition_id_tensor.name if nc.partition_id_tensor else None
        )
        in_names, out_names, out_avals, zero_outs = [], [], [], []
        for alloc in nc.m.functions[0].allocations:
            if not isinstance(alloc, mybir.MemoryLocationSet):
                continue
            name = alloc.memorylocations[0].name
            if alloc.kind == "ExternalInput":
                if name != partition_name:
                    in_names.append(name)
            elif alloc.kind == "ExternalOutput":
                shape = tuple(alloc.tensor_shape)
                dtype = mybir.dt.np(alloc.dtype)
                out_names.append(name)
                out_avals.append(jax.core.ShapedArray(shape, dtype))
                zero_outs.append(np.zeros(shape, dtype))
        self.in_names, self.out_names = in_names, out_names
        self.out_avals, self.zero_outs = out_avals, zero_outs
        n_params, n_outs = len(in_names), len(out_names)
        all_in_names = list(in_names) + list(out_names)
        if partition_name is not None:
            all_in_names.append(partition_name)
        all_in_names = tuple(all_in_names)

        def _body(*args):
            operands = list(args)
            if partition_name is not None:
                operands.append(bass2jax.partition_id_tensor())
            outs = bass2jax._bass_exec_p.bind(
                *operands,
                out_avals=tuple(out_avals),
                in_names=all_in_names,
                out_names=tuple(out_names),
                lowering_input_output_aliases=(),
                sim_require_finite=True,
                sim_require_nnan=True,
                nc=nc,
            )
            return tuple(outs)

        devices = jax.devices()[:N_CORES]
        assert len(devices) == N_CORES
        self.mesh = Mesh(np.asarray(devices), ("core",))
        in_specs = (PartitionSpec("core"),) * (n_params + n_outs)
        out_specs = (PartitionSpec("core"),) * n_outs
        self.donate = tuple(range(n_params, n_params + n_outs))
        self.sharded = jax.jit(
            shard_map(
                _body,
                mesh=self.mesh,
                in_specs=in_specs,
                out_specs=out_specs,
                check_rep=False,
            ),
            donate_argnums=self.donate,
            keep_unused=True,
        )

    def stage_inputs(self, per_core_in):
        """per_core_in: list of dicts (len N_CORES) -> device-resident concat arrays."""
        import jax
        from jax.sharding import NamedSharding, PartitionSpec

        sh = NamedSharding(self.mesh, PartitionSpec("core"))
        staged = []
        for name in self.in_names:
            cat = np.concatenate(
                [np.asarray(per_core_in[c][name]) for c in range(N_CORES)], axis=0
            )
            staged.append(jax.device_put(cat, sh))
        return staged

    def fresh_outs(self):
        import jax
        from jax.sharding import NamedSharding, PartitionSpec

        sh = NamedSharding(self.mesh, PartitionSpec("core"))
        return [
            jax.device_put(
                np.zeros((N_CORES * z.shape[0], *z.shape[1:]), z.dtype), sh
            )
            for z in self.zero_outs
        ]

    def run(self, staged_in, out_bufs):
        import jax

        outs = self.sharded(*staged_in, *out_bufs)
        jax.block_until_ready(outs)
        return outs

    def results(self, outs):
        res = []
        for c in range(N_CORES):
            d = {}
            for i, name in enumerate(self.out_names):
                full = np.asarray(outs[i])
                d[name] = full.reshape(N_CORES, *self.out_avals[i].shape)[c]
            res.append(d)
        return res


def _get_runner(reps=1):
    if reps not in _RUNNERS:
        _RUNNERS[reps] = _Runner(reps)
    return _RUNNERS[reps]


# ---------------------------------------------------------------------------
# Host-side sharding / gather
def _per_core_inputs(x, w_qkv, w_out):
    x = np.asarray(x, dtype=np.float32)
    w_qkv = np.asarray(w_qkv, dtype=np.float32)
    w_out = np.asarray(w_out, dtype=np.float32)
    per_core = []
    for c in range(N_CORES):
        b, g = c // GROUPS, c % GROUPS
        hs = np.arange(g * HPG, (g + 1) * HPG)
        # qkv reshape order in reference: f = d*48 + k*16 + h
        rows_q = (np.arange(D)[None, :] * (3 * H_TOTAL) + hs[:, None]).reshape(-1)
        rows_k = rows_q + H_TOTAL
        rows_v = rows_q + 2 * H_TOTAL
        wqk = np.concatenate([w_qkv[rows_q], SCALE * w_qkv[rows_k]], axis=0)
        per_core.append(
            {
                "xT": np.ascontiguousarray(x[b].T).astype(np.float16),
                "wqkT": np.ascontiguousarray(wqk.T).astype(np.float16),
                "wvT": np.ascontiguousarray(w_qkv[rows_v].T).astype(np.float16),
                "woutT": np.ascontiguousarray(w_out[:, g * DV:(g + 1) * DV].T).astype(np.float16),
            }
        )
    return per_core


def kernel(x, w_qkv, w_out):
    runner = _get_runner(1)
    staged = runner.stage_inputs(_per_core_inputs(x, w_qkv, w_out))
    outs = runner.run(staged, runner.fresh_outs())
    res = runner.results(outs)
    y = np.zeros((B, T, E), dtype=np.float64)
    for c in range(N_CORES):
        y[c // GROUPS] += res[c]["y"].astype(np.float64)
    return y.astype(np.float32)



# revision 15
# speedup vs baseline: 1.4866x; 1.4866x over previous
"""Trainium2 Bass kernel for 16-head MHA (B=2, T=2048, E=1024), SPMD on 8 cores.

Sharding: data-parallel over batch (2) x tensor-parallel over heads (4 groups
of 4 heads). Each core computes, for its (batch b, head-group g):
  qk^T projection (feature-major), v projection (token-major),
  shifted-softmax attention via an augmented-row matmul trick, and a partial
  output projection over its 256 embedding columns. The host sums the 4
  partial projections per batch.

Softmax shift: the exact per-query max over all keys is computed on-device
(q-stationary matmul in [i, j] orientation + free-dim DVE reduces) and folded
into the main QK^T matmul as a rank-1 augmented row, so scores arrive in PSUM
already shifted: S'[j,i] = 8*q_i.k_j - M_i. exp() runs on ACT straight out of
PSUM. The softmax denominator comes for free from a ones-column appended to V.
"""

import sys

sys.path.insert(0, "/opt/trn_rl_repo")

import numpy as np

import concourse.bass as bass
import concourse.mybir as mybir
import concourse.tile as tile_mod
from concourse.masks import make_identity

F32 = mybir.dt.float32
F16 = mybir.dt.float16

B, T, E = 2, 2048, 1024
H_TOTAL, D = 16, 64
N_CORES = 8
GROUPS = 4          # head-group (tensor) parallelism
HPG = H_TOTAL // GROUPS  # 4 heads per group
DV = HPG * D        # 256: v width / out-proj contraction per core
FQK = 2 * DV        # 512: q+k feature rows per core
SCALE = float(np.sqrt(D))  # reference MULTIPLIES scores by sqrt(d)

NE = E // 128       # 8 e-chunks
NT_TILE = T // 128  # 16 token tiles
NT_CHUNK = T // 512  # 4 token chunks
N_SAMPLE_TILES = 2  # 256-key sample for the softmax shift


# ---------------------------------------------------------------------------
# Workaround: this walrus build only accepts ONE sem wait per instruction.
# After Tile scheduling, split every multi-wait instruction: the overflow
# waits move onto same-engine NoOps inserted immediately before it.
def _split_multi_waits(nc):
    for f in nc.m.functions:
        for bb in f.blocks:
            out = []
            for inst in bb.instructions:
                si = getattr(inst, "sync_info", None)
                if si is not None and si.on_wait and len(si.on_wait) > 1:
                    extras = list(si.on_wait[:-1])
                    si.on_wait = list(si.on_wait[-1:])
                    for w in extras:
                        nop = mybir.InstNoOp(
                            name=f"I-{nc.next_id()}", ins=[], outs=[]
                        )
                        nop.engine = inst.engine
                        nop.sync_info = mybir.SyncInfo(on_wait=[w], on_update=[])
                        out.append(nop)
                out.append(inst)
            bb.instructions[:] = out


# ---------------------------------------------------------------------------
# Device program (identical on every core; per-core data differs)
#
# All matmul operands are fp16 and K-padded to the full 128 partitions
# (zero rows) -- K=64/65 matmuls stream rhs columns at half rate on TRN2
# (measured 513ns vs 265ns for N=512), so padding doubles PE throughput.
# Max pass and main pass are software-pipelined per head: while head h runs
# its QK/exp/PV blocks, head h+1's max pass (PE matmuls + DVE reduces)
# weaves between them at 4-matmul granularity, hiding the DVE reduce time.
def _emit_body(nc, tc, dram, ctx_pools, dbg=None):
    xT_d, wqkT_d, wvT_d, woutT_d, y_d = dram
    persist = ctx_pools["persist"]

    # persistent SBUF
    qk_sb = [persist.tile([128, T], F16, tag=f"qk{i}", name=f"qk{i}") for i in range(FQK // 128)]
    # v as one [128, 16*256] tile: [t-tile partition, (jt, dv)] layout
    v_sb = persist.tile([128, NT_TILE * DV], F16, tag="v", name="v")
    oall_sb = [persist.tile([128, T], F16, tag=f"oall{i}", name=f"oall{i}") for i in range(DV // 128)]
    wout_sb = [persist.tile([128, E], F16, tag=f"wout{i}", name=f"wout{i}") for i in range(DV // 128)]
    # one-hot row selectors for the reciprocal broadcast (K=128 matmul):
    # ones_sel[par][den] row = 1, all else 0; junk rows of r are multiplied
    # by zero so only the denominator row is broadcast.
    ones_sel = []
    for par, dn in ((0, D), (1, 0)):
        t = persist.tile([128, D], F16, tag=f"ones{par}", name=f"ones{par}")
        nc.vector.memset(t, 0.0)
        nc.vector.memset(t[dn:dn + 1, :], 1.0)
        ones_sel.append(t)
    identity = persist.tile([128, 128], F32, tag="identity", name="identity")
    make_identity(nc, identity)
    for i in range(DV // 128):
        nc.sync.dma_start(out=wout_sb[i], in_=woutT_d[i * 128:(i + 1) * 128, :])

    # ---- Phase 1: projections ----------------------------------------
    # ff order (0,2,1,3): heads 0/1 need qk tiles 0 and 2 first. PSUM->SBUF
    # copies go via ACT (idle in phase 1) so the DVE is free for aug builds.
    with (
        tc.tile_pool(name="ph1", bufs=1) as ph1,
        tc.tile_pool(name="pj", bufs=4, space="PSUM") as pj,
        tc.tile_pool(name="pv", bufs=2, space="PSUM") as pv,
    ):
        xt_sb = [ph1.tile([128, T], F16, tag=f"xt{i}", name=f"xt{i}") for i in range(NE)]
        wqk_sb = [ph1.tile([128, FQK], F16, tag=f"wqk{i}", name=f"wqk{i}") for i in range(NE)]
        wv_sb = [ph1.tile([128, DV], F16, tag=f"wv{i}", name=f"wv{i}") for i in range(NE)]
        for i in range(NE):
            nc.sync.dma_start(out=wqk_sb[i], in_=wqkT_d[i * 128:(i + 1) * 128, :])
            nc.sync.dma_start(out=xt_sb[i], in_=xT_d[i * 128:(i + 1) * 128, :])
            nc.sync.dma_start(out=wv_sb[i], in_=wvT_d[i * 128:(i + 1) * 128, :])

        # qk^T [f', t] = W'[f', e] @ x^T[e, t], feature-major
        for ff in (0, 2, 1, 3):
            ps = [pj.tile([128, 512], F32, tag="pj", name="pj") for _ in range(NT_CHUNK)]
            for ne in range(NE):
                lhsT = wqk_sb[ne][:, ff * 128:(ff + 1) * 128]
                for tt in range(NT_CHUNK):
                    nc.tensor.matmul(
                        ps[tt],
                        lhsT,
                        xt_sb[ne][:, tt * 512:(tt + 1) * 512],
                        start=(ne == 0),
                        stop=(ne == NE - 1),
                    )
            for tt in range(NT_CHUNK):
                nc.scalar.activation(
                    out=qk_sb[ff][:, tt * 512:(tt + 1) * 512], in_=ps[tt],
                    func=mybir.ActivationFunctionType.Copy,
                )

        # v [t, dv] token-major (full-bank PSUM tiles; half-bank tiles
        # measured pathologically slow)
        for tj in range(NT_TILE):
            psv = pv.tile([128, 512], F32, tag="pv", name="pv")
            for ne in range(NE):
                nc.tensor.matmul(
                    psv[:, 0:DV],
                    xt_sb[ne][:, tj * 128:(tj + 1) * 128],
                    wv_sb[ne],
                    start=(ne == 0),
                    stop=(ne == NE - 1),
                )
            nc.scalar.activation(
                out=v_sb[:, tj * DV:(tj + 1) * DV], in_=psv[:, 0:DV],
                func=mybir.ActivationFunctionType.Copy,
            )

    # ---- Phase 2: attention -------------------------------------------
    # Partition-alignment rules (DVE/ACT lanes are hardwired per partition):
    # odd heads' q/k rows live at SBUF partitions 64..127 of qk_sb, so the
    # copy into base-0 aug tiles goes via DMA (address-based, can shift
    # partitions). The per-head V-stationary tile [128, jt*128] puts the
    # ones column and O rows at parity-dependent positions so every later
    # DVE op sees matching base partitions.
    with (
        tc.tile_pool(name="aug", bufs=1) as augp,
        tc.tile_pool(name="vaug", bufs=1) as vaugp,
    ):
        k_augs, q_augs, vaugs = [], [], []
        v_r = v_sb.rearrange("p (j c) -> p j c", c=DV)
        for h in range(HPG):
            odd = h % 2 == 1
            q_tile, k_tile = h // 2, 2 + h // 2
            # [128, T]: rows 0:64 = q|k, row 64 = -M slot (q, zero-init) /
            # ones (k), rows 65:128 = zero K-padding.
            k_aug = augp.tile([128, T], F16, tag=f"kaug{h}", name=f"kaug{h}")
            q_aug = augp.tile([128, T], F16, tag=f"qaug{h}", name=f"qaug{h}")
            if odd:
                nc.sync.dma_start(out=k_aug[0:D, :], in_=qk_sb[k_tile][D:2 * D, :])
                nc.sync.dma_start(out=q_aug[0:D, :], in_=qk_sb[q_tile][D:2 * D, :])
            else:
                nc.vector.tensor_copy(out=k_aug[0:D, :], in_=qk_sb[k_tile][0:D, :])
                nc.vector.tensor_copy(out=q_aug[0:D, :], in_=qk_sb[q_tile][0:D, :])
            nc.vector.memset(q_aug[D:128, :], 0.0)
            nc.vector.memset(k_aug[D:128, :], 0.0)
            nc.vector.memset(k_aug[D:D + 1, :], 1.0)
            k_augs.append(k_aug)
            q_augs.append(q_aug)
        for h in range(HPG):
            odd = h % 2 == 1
            # V-stationary [128, 16*128]: per j-tile 128 columns holding
            # (even) [v(64) | ones | 0...]   -> O rows 0..63, denom row 64
            # (odd)  [ones | 0... | v(64)]   -> denom row 0, O rows 64..127
            vaug = vaugp.tile([128, NT_TILE * 128], F16, tag=f"vaug{h}", name=f"vaug{h}")
            va_r = vaug.rearrange("p (j c) -> p j c", c=128)
            eng = nc.gpsimd if odd else nc.vector
            if odd:
                eng.memset(va_r[:, :, 0:1], 1.0)
                eng.memset(va_r[:, :, 1:D], 0.0)
                eng.tensor_copy(out=va_r[:, :, D:2 * D], in_=v_r[:, :, h * D:(h + 1) * D])
            else:
                eng.tensor_copy(out=va_r[:, :, 0:D], in_=v_r[:, :, h * D:(h + 1) * D])
                eng.memset(va_r[:, :, D:D + 1], 1.0)
                eng.memset(va_r[:, :, D + 1:128], 0.0)
            vaugs.append(vaug)

        with (
            tc.tile_pool(name="mx", bufs=1) as mxp,
            tc.tile_pool(name="pt", bufs=4) as ptp,
            tc.tile_pool(name="rr", bufs=2) as rrp,
            tc.tile_pool(name="ot", bufs=2) as otp,
            tc.tile_pool(name="ps", bufs=2, space="PSUM") as psp,
            tc.tile_pool(name="po", bufs=2, space="PSUM") as pop,
            tc.tile_pool(name="pr", bufs=1, space="PSUM") as prp,
            tc.tile_pool(name="px", bufs=2, space="PSUM") as pxp,
            tc.tile_pool(name="mx4", bufs=1, space="PSUM") as mx4p,
        ):
            # per-head persistent r tiles, zeroed once: the broadcast
            # matmul streams all 128 rows of r, so the 0-weighted rows must
            # hold finite values; only the denominator row is ever rewritten.
            r_tiles = []
            for h in range(HPG):
                rt = rrp.tile([128, 512], F16, tag=f"r{h}", name=f"r{h}", bufs=1)
                nc.vector.memset(rt, 0.0)
                r_tiles.append(rt)

            # ---- max pass emitters ------------------------------------
            # scores in [i-part, j-free] orientation (q-stationary, K=128
            # zero-padded); row max = DVE free-axis reduce; combines and
            # negate on GpSimd (SBUF-only engine); PE transpose lays -M
            # along the free dim of q_aug row D via one small DMA.
            max_state = {}

            def max_quad(h, ic, c):
                q_aug, k_aug = q_augs[h], k_augs[h]
                if c == 0:
                    max_state[(h, ic)] = (
                        mxp.tile([128, NT_CHUNK], F32, tag="mxt", name="mxt", bufs=2),
                        mxp.tile([128, NT_CHUNK * 3], F32, tag="mxt2", name="mxt2", bufs=2),
                    )
                mxt, mxt2 = max_state[(h, ic)]
                m2r = mxt2.rearrange("p (c j) -> p c j", j=3)
                it = ic * NT_CHUNK + c
                for jc in range(NT_CHUNK):
                    s_ps = pxp.tile([128, 512], F32, tag="px", name="px")
                    nc.tensor.matmul(
                        s_ps,
                        q_aug[:, it * 128:(it + 1) * 128],
                        k_aug[:, jc * 512:(jc + 1) * 512],
                        start=True,
                        stop=True,
                    )
                    if jc == 0:
                        nc.vector.reduce_max(
                            out=mxt[:, c:c + 1], in_=s_ps,
                            axis=mybir.AxisListType.X,
                        )
                    else:
                        nc.vector.reduce_max(
                            out=m2r[:, c, jc - 1:jc], in_=s_ps,
                            axis=mybir.AxisListType.X,
                        )

            def max_finish(h, ic):
                q_aug = q_augs[h]
                ics = slice(ic * 512, (ic + 1) * 512)
                mxt, mxt2 = max_state.pop((h, ic))
                m2r = mxt2.rearrange("p (c j) -> p c j", j=3)
                mc = mxp.tile([128, NT_CHUNK], F32, tag="mc", name="mc", bufs=2)
                nc.vector.tensor_max(mc, m2r[:, :, 0], m2r[:, :, 1])
                nc.vector.tensor_max(mc, mc, m2r[:, :, 2])
                nc.vector.tensor_max(mxt, mxt, mc)
                mx4 = mx4p.tile([NT_CHUNK, 128], F32, tag="mx4", name="mx4")
                nc.tensor.transpose(mx4, mxt, identity)
                mrow = mxp.tile([NT_CHUNK, 128], F16, tag="mrow", name="mrow", bufs=2)
                nc.vector.tensor_scalar_mul(mrow, mx4, -1.0)
                nc.sync.dma_start(
                    out=q_aug[D:D + 1, ics].rearrange(
                        "p (c f) -> p c f", c=NT_CHUNK
                    ),
                    in_=mrow,
                )

            # ---- main-pass emitters -----------------------------------
            pending = []  # delayed (PE replicate + DVE mul) closures

            def flush_pending():
                while pending:
                    pending.pop(0)()

            def main_block(h, ic, weave_h=None):
                # weave_h: head whose max pass interleaves with this block
                odd = h % 2 == 1
                o_base = (h % 2) * D
                den = D if not odd else 0
                ics = slice(ic * 512, (ic + 1) * 512)
                q_aug, k_aug, vaug = q_augs[h], k_augs[h], vaugs[h]
                po = pop.tile([128, 512], F32, tag="po", name="po")
                for jq in range(NT_CHUNK):
                    if weave_h is not None:
                        max_quad(weave_h, ic, jq)
                    for jt in range(jq * NT_CHUNK, (jq + 1) * NT_CHUNK):
                        ps = psp.tile([128, 512], F32, tag="ps", name="ps")
                        nc.tensor.matmul(
                            ps,
                            k_aug[:, jt * 128:(jt + 1) * 128],
                            q_aug[:, ics],
                            start=True,
                            stop=True,
                        )
                        pT = ptp.tile([128, 512], F16, tag="pt", name="pt")
                        nc.scalar.activation(
                            out=pT, in_=ps, func=mybir.ActivationFunctionType.Exp
                        )
                        nc.tensor.matmul(
                            po,
                            vaug[:, jt * 128:(jt + 1) * 128],
                            pT,
                            start=(jt == 0),
                            stop=(jt == NT_TILE - 1),
                        )
                if weave_h is not None:
                    max_finish(weave_h, ic)
                flush_pending()
                # eager: free po's bank fast (reciprocal on DVE via the
                # ~18-bit fast approx, staging copy on ACT)
                r = r_tiles[h]
                nc.vector.reciprocal(out=r[den:den + 1, :], in_=po[den:den + 1, :])
                ot = otp.tile([128, 512], F32, tag="ot", name="ot")
                nc.scalar.activation(
                    out=ot[o_base:o_base + D, :], in_=po[o_base:o_base + D, :],
                    func=mybir.ActivationFunctionType.Copy,
                )

                def normalize():
                    # delayed: PE broadcast of r across O rows + DVE multiply
                    pr = prp.tile([128, 512], F32, tag="pr", name="pr")
                    nc.tensor.matmul(
                        pr[o_base:o_base + D, :],
                        ones_sel[h % 2][:, 0:D],
                        r,
                        start=True,
                        stop=True,
                    )
                    nc.vector.tensor_mul(
                        oall_sb[h // 2][o_base:o_base + D, ics],
                        ot[o_base:o_base + D, :],
                        pr[o_base:o_base + D, :],
                    )

                pending.append(normalize)

            # ---- schedule: head-staggered max/main --------------------
            for ic in range(NT_CHUNK):
                for c in range(NT_CHUNK):
                    max_quad(0, ic, c)
                max_finish(0, ic)
            for h in range(HPG):
                for ic in range(NT_CHUNK):
                    main_block(h, ic, weave_h=h + 1 if h + 1 < HPG else None)
            flush_pending()

    # ---- Phase 3: partial out-projection ------------------------------
    with (
        tc.tile_pool(name="ysb", bufs=3) as ysbp,
        tc.tile_pool(name="py", bufs=4, space="PSUM") as pyp,
    ):
        for tt in range(NT_TILE):
            pys = [pyp.tile([128, 512], F32, tag="py", name="py") for _ in range(2)]
            for es in range(DV // 128):
                lhsT = oall_sb[es][:, tt * 128:(tt + 1) * 128]
                for oc in range(2):
                    nc.tensor.matmul(
                        pys[oc],
                        lhsT,
                        wout_sb[es][:, oc * 512:(oc + 1) * 512],
                        start=(es == 0),
                        stop=(es == DV // 128 - 1),
                    )
            yt = ysbp.tile([128, E], F32, tag="y", name="y")
            for oc in range(2):
                nc.vector.tensor_copy(out=yt[:, oc * 512:(oc + 1) * 512], in_=pys[oc])
            nc.sync.dma_start(out=y_d[tt * 128:(tt + 1) * 128, :], in_=yt)


def _build_nc(reps=1, debug=False, split_waits=True):
    nc = bass.Bass()
    xT_d = nc.declare_dram_parameter("xT", [E, T], F16, isOutput=False)
    wqkT_d = nc.declare_dram_parameter("wqkT", [E, FQK], F16, isOutput=False)
    wvT_d = nc.declare_dram_parameter("wvT", [E, DV], F16, isOutput=False)
    woutT_d = nc.declare_dram_parameter("woutT", [DV, E], F16, isOutput=False)
    y_d = nc.declare_dram_parameter("y", [T, E], F32, isOutput=True)
    dram = (xT_d, wqkT_d, wvT_d, woutT_d, y_d)
    dbg = None
    if debug:
        shapes = {
            "qk0": [128, T],
            "qk2": [128, T],
            "v0": [128, NT_TILE * DV],
            "qaug0": [D + 1, T],
            "kaug0": [D + 1, T],
            "vaug0": [128, NT_TILE * 128],
            "oall0": [128, T],
            "den0": [NT_CHUNK, 512],
            "sp0": [128, T],
        }
        keys = debug if isinstance(debug, (list, tuple)) else list(shapes)
        dbg = {
            k: nc.declare_dram_parameter(k, shapes[k], F32, isOutput=True)
            for k in keys
        }
    with tile_mod.TileContext(nc) as tc, nc.allow_low_precision(
        reason="fp16 kernel: scores/softmax accumulate in fp32 PSUM; fp16 "
        "elsewhere is validated to rel-err ~5e-3 vs the fp64 reference"
    ):
        for _ in range(reps):
            with tc.tile_pool(name="persist", bufs=1) as persist:
                _emit_body(nc, tc, dram, {"persist": persist}, dbg=dbg)
    if split_waits:
        _split_multi_waits(nc)
    return nc


# ---------------------------------------------------------------------------
# Execution: cached jitted shard_map over 8 cores (axon/PJRT path)
_RUNNERS = {}


class _Runner:
    def __init__(self, reps=1, debug=False):
        import jax
        from jax.sharding import Mesh, PartitionSpec
        from jax.experimental.shard_map import shard_map
        from concourse import bass2jax

        bass2jax.install_neuronx_cc_hook()
        nc = self._nc = _build_nc(reps, debug=debug)

        partition_name = (
            nc.partition_id_tensor.name if nc.partition_id_tensor else None
        )
        in_names, out_names, out_avals, zero_outs = [], [], [], []
        for alloc in nc.m.functions[0].allocations:
            if not isinstance(alloc, mybir.MemoryLocationSet):
                continue
            name = alloc.memorylocations[0].name
            if alloc.kind == "ExternalInput":
                if name != partition_name:
                    in_names.append(name)
            elif alloc.kind == "ExternalOutput":
                shape = tuple(alloc.tensor_shape)
                dtype = mybir.dt.np(alloc.dtype)
                out_names.append(name)
                out_avals.append(jax.core.ShapedArray(shape, dtype))
                zero_outs.append(np.zeros(shape, dtype))
        self.in_names, self.out_names = in_names, out_names
        self.out_avals, self.zero_outs = out_avals, zero_outs
        n_params, n_outs = len(in_names), len(out_names)
        all_in_names = list(in_names) + list(out_names)
        if partition_name is not None:
            all_in_names.append(partition_name)
        all_in_names = tuple(all_in_names)

        def _body(*args):
            operands = list(args)
            if partition_name is not None:
                operands.append(bass2jax.partition_id_tensor())
            outs = bass2jax._bass_exec_p.bind(
                *operands,
                out_avals=tuple(out_avals),
                in_names=all_in_names,
                out_names=tuple(out_names),
                lowering_input_output_aliases=(),
                sim_require_finite=True,
                sim_require_nnan=True,
                nc=nc,
            )
            return tuple(outs)

        devices = jax.devices()[:N_CORES]
        assert len(devices) == N_CORES
        self.mesh = Mesh(np.asarray(devices), ("core",))
        in_specs = (PartitionSpec("core"),) * (n_params + n_outs)
        out_specs = (PartitionSpec("core"),) * n_outs
        self.donate = tuple(range(n_params, n_params + n_outs))
        self.sharded = jax.jit(
            shard_map(
                _body,
                mesh=self.mesh,
                in_specs=in_specs,
                out_specs=out_specs,
                check_rep=False,
            ),
            donate_argnums=self.donate,
            keep_unused=True,
        )

    def stage_inputs(self, per_core_in):
        """per_core_in: list of dicts (len N_CORES) -> device-resident concat arrays."""
        import jax
        from jax.sharding import NamedSharding, PartitionSpec

        sh = NamedSharding(self.mesh, PartitionSpec("core"))
        staged = []
        for name in self.in_names:
            cat = np.concatenate(
                [np.asarray(per_core_in[c][name]) for c in range(N_CORES)], axis=0
            )
            staged.append(jax.device_put(cat, sh))
        return staged

    def fresh_outs(self):
        import jax
        from jax.sharding import NamedSharding, PartitionSpec

        sh = NamedSharding(self.mesh, PartitionSpec("core"))
        return [
            jax.device_put(
                np.zeros((N_CORES * z.shape[0], *z.shape[1:]), z.dtype), sh
            )
            for z in self.zero_outs
        ]

    def run(self, staged_in, out_bufs):
        import jax

        outs = self.sharded(*staged_in, *out_bufs)
        jax.block_until_ready(outs)
        return outs

    def results(self, outs):
        res = []
        for c in range(N_CORES):
            d = {}
            for i, name in enumerate(self.out_names):
                full = np.asarray(outs[i])
                d[name] = full.reshape(N_CORES, *self.out_avals[i].shape)[c]
            res.append(d)
        return res


def _get_runner(reps=1):
    if reps not in _RUNNERS:
        _RUNNERS[reps] = _Runner(reps)
    return _RUNNERS[reps]


# ---------------------------------------------------------------------------
# Host-side sharding / gather
def _per_core_inputs(x, w_qkv, w_out):
    x = np.asarray(x, dtype=np.float32)
    w_qkv = np.asarray(w_qkv, dtype=np.float32)
    w_out = np.asarray(w_out, dtype=np.float32)
    per_core = []
    for c in range(N_CORES):
        b, g = c // GROUPS, c % GROUPS
        hs = np.arange(g * HPG, (g + 1) * HPG)
        # qkv reshape order in reference: f = d*48 + k*16 + h
        rows_q = (np.arange(D)[None, :] * (3 * H_TOTAL) + hs[:, None]).reshape(-1)
        rows_k = rows_q + H_TOTAL
        rows_v = rows_q + 2 * H_TOTAL
        wqk = np.concatenate([w_qkv[rows_q], SCALE * w_qkv[rows_k]], axis=0)
        per_core.append(
            {
                "xT": np.ascontiguousarray(x[b].T).astype(np.float16),
                "wqkT": np.ascontiguousarray(wqk.T).astype(np.float16),
                "wvT": np.ascontiguousarray(w_qkv[rows_v].T).astype(np.float16),
                "woutT": np.ascontiguousarray(w_out[:, g * DV:(g + 1) * DV].T).astype(np.float16),
            }
        )
    return per_core


def kernel(x, w_qkv, w_out):
    runner = _get_runner(1)
    staged = runner.stage_inputs(_per_core_inputs(x, w_qkv, w_out))
    outs = runner.run(staged, runner.fresh_outs())
    res = runner.results(outs)
    y = np.zeros((B, T, E), dtype=np.float64)
    for c in range(N_CORES):
        y[c // GROUPS] += res[c]["y"].astype(np.float64)
    return y.astype(np.float32)



# revision 16
# speedup vs baseline: 2.0592x; 1.3851x over previous
"""Trainium2 Bass kernel for 16-head MHA (B=2, T=2048, E=1024), SPMD on 8 cores.

Sharding: data-parallel over batch (2) x tensor-parallel over heads (4 groups
of 4 heads). Each core computes, for its (batch b, head-group g):
  qk^T projection (feature-major), v projection (token-major),
  shifted-softmax attention via an augmented-row matmul trick, and a partial
  output projection over its 256 embedding columns. The host sums the 4
  partial projections per batch.

Softmax shift: the exact per-query max over all keys is computed on-device
(q-stationary matmul in [i, j] orientation + free-dim DVE reduces) and folded
into the main QK^T matmul as a rank-1 augmented row, so scores arrive in PSUM
already shifted: S'[j,i] = 8*q_i.k_j - M_i. exp() runs on ACT straight out of
PSUM. The softmax denominator comes for free from a ones-column appended to V.
"""

import sys

sys.path.insert(0, "/opt/trn_rl_repo")

import numpy as np

import concourse.bass as bass
import concourse.mybir as mybir
import concourse.tile as tile_mod
from concourse.masks import make_identity

F32 = mybir.dt.float32
F16 = mybir.dt.float16

B, T, E = 2, 2048, 1024
H_TOTAL, D = 16, 64
N_CORES = 8
GROUPS = 4          # head-group (tensor) parallelism
HPG = H_TOTAL // GROUPS  # 4 heads per group
DV = HPG * D        # 256: v width / out-proj contraction per core
FQK = 2 * DV        # 512: q+k feature rows per core
SCALE = float(np.sqrt(D))  # reference MULTIPLIES scores by sqrt(d)

NE = E // 128       # 8 e-chunks
NT_TILE = T // 128  # 16 token tiles
NT_CHUNK = T // 512  # 4 token chunks
N_SAMPLE_TILES = 2  # 256-key sample for the softmax shift


# ---------------------------------------------------------------------------
# Workaround: this walrus build only accepts ONE sem wait per instruction.
# After Tile scheduling, split every multi-wait instruction: the overflow
# waits move onto same-engine NoOps inserted immediately before it.
def _split_multi_waits(nc):
    for f in nc.m.functions:
        for bb in f.blocks:
            out = []
            for inst in bb.instructions:
                si = getattr(inst, "sync_info", None)
                if si is not None and si.on_wait and len(si.on_wait) > 1:
                    extras = list(si.on_wait[:-1])
                    si.on_wait = list(si.on_wait[-1:])
                    for w in extras:
                        nop = mybir.InstNoOp(
                            name=f"I-{nc.next_id()}", ins=[], outs=[]
                        )
                        nop.engine = inst.engine
                        nop.sync_info = mybir.SyncInfo(on_wait=[w], on_update=[])
                        out.append(nop)
                out.append(inst)
            bb.instructions[:] = out


# ---------------------------------------------------------------------------
# Device program (identical on every core; per-core data differs)
#
# All matmul operands are fp16 and K-padded to the full 128 partitions
# (zero rows) -- K=64/65 matmuls stream rhs columns at half rate on TRN2
# (measured 513ns vs 265ns for N=512), so padding doubles PE throughput.
# Scheduling: head h+1's max pass (PE matmuls + DVE reduces) weaves through
# head h's QK/exp/PV blocks; head 0's max pass weaves through the phase-1
# projections; the out-projection for chunk ic weaves through head 3's
# block ic+1. The per-head softmax denominator reciprocal runs on ACT as
# exp(-ln(d)) so the slow single-partition DVE reciprocal never gates the
# PE's broadcast matmul.
def _emit_body(nc, tc, dram, ctx_pools, dbg=None):
    xT_d, wqkT_d, wvT_d, woutT_d, y_d = dram
    persist = ctx_pools["persist"]

    # persistent SBUF
    qk_sb = [persist.tile([128, T], F16, tag=f"qk{i}", name=f"qk{i}") for i in range(FQK // 128)]
    # v as one [128, 16*256] tile: [t-tile partition, (jt, dv)] layout
    v_sb = persist.tile([128, NT_TILE * DV], F16, tag="v", name="v")
    oall_sb = [persist.tile([128, T], F16, tag=f"oall{i}", name=f"oall{i}") for i in range(DV // 128)]
    wout_sb = [persist.tile([128, E], F16, tag=f"wout{i}", name=f"wout{i}") for i in range(DV // 128)]
    # one-hot row selectors for the reciprocal broadcast (K=128 matmul):
    # ones_sel[par][den] row = 1, all else 0; junk rows of r are multiplied
    # by zero so only the denominator row is broadcast.
    ones_sel = []
    for par, dn in ((0, D), (1, 0)):
        t = persist.tile([128, D], F16, tag=f"ones{par}", name=f"ones{par}")
        nc.vector.memset(t, 0.0)
        nc.vector.memset(t[dn:dn + 1, :], 1.0)
        ones_sel.append(t)
    identity = persist.tile([128, 128], F32, tag="identity", name="identity")
    make_identity(nc, identity)
    for i in range(DV // 128):
        nc.sync.dma_start(out=wout_sb[i], in_=woutT_d[i * 128:(i + 1) * 128, :])

    with (
        tc.tile_pool(name="aug", bufs=1) as augp,
        tc.tile_pool(name="vaug", bufs=1) as vaugp,
        tc.tile_pool(name="mx", bufs=1) as mxp,
        tc.tile_pool(name="px", bufs=2, space="PSUM") as pxp,
        tc.tile_pool(name="mx4", bufs=1, space="PSUM") as mx4p,
    ):
        k_augs, q_augs, vaugs = [], [], []

        def build_qk_aug(h):
            # [128, T]: rows 0:64 = q|k, row 64 = -M slot (q, zero-init) /
            # ones (k), rows 65:128 = zero K-padding. Odd heads' rows live
            # at partitions 64..127 of qk_sb -> partition-shifting DMA.
            odd = h % 2 == 1
            q_tile, k_tile = h // 2, 2 + h // 2
            k_aug = augp.tile([128, T], F16, tag=f"kaug{h}", name=f"kaug{h}")
            q_aug = augp.tile([128, T], F16, tag=f"qaug{h}", name=f"qaug{h}")
            if odd:
                nc.sync.dma_start(out=k_aug[0:D, :], in_=qk_sb[k_tile][D:2 * D, :])
                nc.sync.dma_start(out=q_aug[0:D, :], in_=qk_sb[q_tile][D:2 * D, :])
            else:
                nc.vector.tensor_copy(out=k_aug[0:D, :], in_=qk_sb[k_tile][0:D, :])
                nc.vector.tensor_copy(out=q_aug[0:D, :], in_=qk_sb[q_tile][0:D, :])
            nc.vector.memset(q_aug[D:128, :], 0.0)
            nc.vector.memset(k_aug[D:128, :], 0.0)
            nc.vector.memset(k_aug[D:D + 1, :], 1.0)
            k_augs.append(k_aug)
            q_augs.append(q_aug)

        # ---- max-pass emitters ----------------------------------------
        # scores in [i-part, j-free] orientation (q-stationary, K=128
        # zero-padded); row max = DVE free-axis reduce; PE transpose lays
        # -M along the free dim of q_aug row D via one small DMA.
        max_state = {}

        def max_quad(h, ic, c):
            q_aug, k_aug = q_augs[h], k_augs[h]
            if c == 0:
                max_state[(h, ic)] = (
                    mxp.tile([128, NT_CHUNK], F32, tag="mxt", name="mxt", bufs=2),
                    mxp.tile([128, NT_CHUNK * 3], F32, tag="mxt2", name="mxt2", bufs=2),
                )
            mxt, mxt2 = max_state[(h, ic)]
            m2r = mxt2.rearrange("p (c j) -> p c j", j=3)
            it = ic * NT_CHUNK + c
            for jc in range(NT_CHUNK):
                s_ps = pxp.tile([128, 512], F32, tag="px", name="px")
                nc.tensor.matmul(
                    s_ps,
                    q_aug[:, it * 128:(it + 1) * 128],
                    k_aug[:, jc * 512:(jc + 1) * 512],
                    start=True,
                    stop=True,
                )
                if jc == 0:
                    nc.vector.reduce_max(
                        out=mxt[:, c:c + 1], in_=s_ps,
                        axis=mybir.AxisListType.X,
                    )
                else:
                    nc.vector.reduce_max(
                        out=m2r[:, c, jc - 1:jc], in_=s_ps,
                        axis=mybir.AxisListType.X,
                    )

        def max_finish(h, ic):
            q_aug = q_augs[h]
            ics = slice(ic * 512, (ic + 1) * 512)
            mxt, mxt2 = max_state.pop((h, ic))
            m2r = mxt2.rearrange("p (c j) -> p c j", j=3)
            mc = mxp.tile([128, NT_CHUNK], F32, tag="mc", name="mc", bufs=2)
            nc.vector.tensor_max(mc, m2r[:, :, 0], m2r[:, :, 1])
            nc.vector.tensor_max(mc, mc, m2r[:, :, 2])
            nc.vector.tensor_max(mxt, mxt, mc)
            mx4 = mx4p.tile([NT_CHUNK, 128], F32, tag="mx4", name="mx4")
            nc.tensor.transpose(mx4, mxt, identity)
            mrow = mxp.tile([NT_CHUNK, 128], F16, tag="mrow", name="mrow", bufs=2)
            nc.vector.tensor_scalar_mul(mrow, mx4, -1.0)
            nc.sync.dma_start(
                out=q_aug[D:D + 1, ics].rearrange(
                    "p (c f) -> p c f", c=NT_CHUNK
                ),
                in_=mrow,
            )

        def max_chunk(h, ic):
            for c in range(NT_CHUNK):
                max_quad(h, ic, c)
            max_finish(h, ic)

        # ---- Phase 1: projections, with head 0's max pass woven in ----
        # ff order (0,2,1,3): heads 0/1 need qk tiles 0 and 2 first.
        # PSUM->SBUF copies go via ACT so the DVE is free for aug builds
        # and max reduces.
        with (
            tc.tile_pool(name="ph1", bufs=1) as ph1,
            tc.tile_pool(name="pj", bufs=4, space="PSUM") as pj,
            tc.tile_pool(name="pv", bufs=1, space="PSUM") as pv,
        ):
            xt_sb = [ph1.tile([128, T], F16, tag=f"xt{i}", name=f"xt{i}") for i in range(NE)]
            wqk_sb = [ph1.tile([128, FQK], F16, tag=f"wqk{i}", name=f"wqk{i}") for i in range(NE)]
            wv_sb = [ph1.tile([128, DV], F16, tag=f"wv{i}", name=f"wv{i}") for i in range(NE)]
            for i in range(NE):
                nc.sync.dma_start(out=wqk_sb[i], in_=wqkT_d[i * 128:(i + 1) * 128, :])
                nc.sync.dma_start(out=xt_sb[i], in_=xT_d[i * 128:(i + 1) * 128, :])
                nc.sync.dma_start(out=wv_sb[i], in_=wvT_d[i * 128:(i + 1) * 128, :])

            def qk_proj(ff):
                # qk^T [f', t] = W'[f', e] @ x^T[e, t], feature-major
                ps = [pj.tile([128, 512], F32, tag="pj", name="pj") for _ in range(NT_CHUNK)]
                for ne in range(NE):
                    lhsT = wqk_sb[ne][:, ff * 128:(ff + 1) * 128]
                    for tt in range(NT_CHUNK):
                        nc.tensor.matmul(
                            ps[tt],
                            lhsT,
                            xt_sb[ne][:, tt * 512:(tt + 1) * 512],
                            start=(ne == 0),
                            stop=(ne == NE - 1),
                        )
                for tt in range(NT_CHUNK):
                    nc.scalar.activation(
                        out=qk_sb[ff][:, tt * 512:(tt + 1) * 512], in_=ps[tt],
                        func=mybir.ActivationFunctionType.Copy,
                    )

            qk_proj(0)
            qk_proj(2)
            build_qk_aug(0)
            build_qk_aug(1)
            qk_proj(1)
            max_chunk(0, 0)
            qk_proj(3)
            build_qk_aug(2)
            build_qk_aug(3)
            max_chunk(0, 1)

            # v [t, dv] token-major, two tiles per full-bank PSUM buffer
            # (half-bank tiles measured pathologically slow); head 0's
            # remaining max chunks weave between pairs.
            for tp in range(NT_TILE // 2):
                psv = pv.tile([128, 512], F32, tag="pv", name="pv")
                for half in range(2):
                    tj = tp * 2 + half
                    for ne in range(NE):
                        nc.tensor.matmul(
                            psv[:, half * DV:(half + 1) * DV],
                            xt_sb[ne][:, tj * 128:(tj + 1) * 128],
                            wv_sb[ne],
                            start=(ne == 0),
                            stop=(ne == NE - 1),
                        )
                nc.scalar.activation(
                    out=v_sb[:, tp * 2 * DV:(tp + 1) * 2 * DV], in_=psv,
                    func=mybir.ActivationFunctionType.Copy,
                )
                if tp == 3:
                    max_chunk(0, 2)
                elif tp == 6:
                    max_chunk(0, 3)

        for h in range(HPG):
            odd = h % 2 == 1
            # V-stationary [128, 16*128]: per j-tile 128 columns holding
            # (even) [v(64) | ones | 0...]   -> O rows 0..63, denom row 64
            # (odd)  [ones | 0... | v(64)]   -> denom row 0, O rows 64..127
            vaug = vaugp.tile([128, NT_TILE * 128], F16, tag=f"vaug{h}", name=f"vaug{h}")
            va_r = vaug.rearrange("p (j c) -> p j c", c=128)
            v_r = v_sb.rearrange("p (j c) -> p j c", c=DV)
            eng = nc.gpsimd if odd else nc.vector
            if odd:
                eng.memset(va_r[:, :, 0:1], 1.0)
                eng.memset(va_r[:, :, 1:D], 0.0)
                eng.tensor_copy(out=va_r[:, :, D:2 * D], in_=v_r[:, :, h * D:(h + 1) * D])
            else:
                eng.tensor_copy(out=va_r[:, :, 0:D], in_=v_r[:, :, h * D:(h + 1) * D])
                eng.memset(va_r[:, :, D:D + 1], 1.0)
                eng.memset(va_r[:, :, D + 1:128], 0.0)
            vaugs.append(vaug)

        # ---- Phase 2 main + phase 3 ------------------------------------
        with (
            tc.tile_pool(name="pt", bufs=4) as ptp,
            tc.tile_pool(name="rr", bufs=1) as rrp,
            tc.tile_pool(name="ot", bufs=2) as otp,
            tc.tile_pool(name="ysb", bufs=3) as ysbp,
            tc.tile_pool(name="ps", bufs=2, space="PSUM") as psp,
            tc.tile_pool(name="po", bufs=2, space="PSUM") as pop,
            tc.tile_pool(name="pr", bufs=1, space="PSUM") as prp,
        ):
            # per-head persistent r tiles, zeroed once: the broadcast
            # matmul streams all 128 rows of r, so the 0-weighted rows must
            # hold finite values; only the denominator row is ever rewritten.
            r_tiles = []
            for h in range(HPG):
                rt = rrp.tile([128, 512], F16, tag=f"r{h}", name=f"r{h}", bufs=1)
                nc.vector.memset(rt, 0.0)
                r_tiles.append(rt)

            pending = []  # delayed (PE broadcast + DVE mul) closures

            def flush_pending():
                while pending:
                    pending.pop(0)()

            def phase3(ic):
                # out-projection for a finished token chunk; PSUM comes
                # from the (idle by now) max-pass pool pxp.
                for tt in range(ic * NT_CHUNK, (ic + 1) * NT_CHUNK):
                    pys = [pxp.tile([128, 512], F32, tag="px", name="px") for _ in range(2)]
                    for es in range(DV // 128):
                        lhsT = oall_sb[es][:, tt * 128:(tt + 1) * 128]
                        for oc in range(2):
                            nc.tensor.matmul(
                                pys[oc],
                                lhsT,
                                wout_sb[es][:, oc * 512:(oc + 1) * 512],
                                start=(es == 0),
                                stop=(es == DV // 128 - 1),
                            )
                    yt = ysbp.tile([128, E], F32, tag="y", name="y")
                    for oc in range(2):
                        nc.vector.tensor_copy(out=yt[:, oc * 512:(oc + 1) * 512], in_=pys[oc])
                    nc.sync.dma_start(out=y_d[tt * 128:(tt + 1) * 128, :], in_=yt)

            def main_block(h, ic, weave_h=None, weave_p3=None):
                odd = h % 2 == 1
                o_base = (h % 2) * D
                den = D if not odd else 0
                ics = slice(ic * 512, (ic + 1) * 512)
                q_aug, k_aug, vaug = q_augs[h], k_augs[h], vaugs[h]
                po = pop.tile([128, 512], F32, tag="po", name="po")
                for jq in range(NT_CHUNK):
                    if weave_h is not None:
                        max_quad(weave_h, ic, jq)
                    for jt in range(jq * NT_CHUNK, (jq + 1) * NT_CHUNK):
                        ps = psp.tile([128, 512], F32, tag="ps", name="ps")
                        nc.tensor.matmul(
                            ps,
                            k_aug[:, jt * 128:(jt + 1) * 128],
                            q_aug[:, ics],
                            start=True,
                            stop=True,
                        )
                        pT = ptp.tile([128, 512], F16, tag="pt", name="pt")
                        nc.scalar.activation(
                            out=pT, in_=ps, func=mybir.ActivationFunctionType.Exp
                        )
                        nc.tensor.matmul(
                            po,
                            vaug[:, jt * 128:(jt + 1) * 128],
                            pT,
                            start=(jt == 0),
                            stop=(jt == NT_TILE - 1),
                        )
                if weave_h is not None:
                    max_finish(weave_h, ic)
                flush_pending()
                if weave_p3 is not None:
                    phase3(weave_p3)
                # eager: free po's bank fast. Reciprocal on ACT as
                # exp(-ln(d)) -- the exact DVE reciprocal is a serial
                # 3.35us op that gated the broadcast matmul.
                r = r_tiles[h]
                lnt = otp.tile([128, 512], F32, tag="lnt", name="lnt", bufs=2)
                nc.scalar.activation(
                    out=lnt[den:den + 1, :], in_=po[den:den + 1, :],
                    func=mybir.ActivationFunctionType.Ln,
                )
                nc.scalar.activation(
                    out=r[den:den + 1, :], in_=lnt[den:den + 1, :],
                    func=mybir.ActivationFunctionType.Exp, scale=-1.0,
                )
                ot = otp.tile([128, 512], F32, tag="ot", name="ot")
                nc.vector.tensor_copy(
                    out=ot[o_base:o_base + D, :], in_=po[o_base:o_base + D, :]
                )

                def normalize():
                    # delayed: PE broadcast of r across O rows + DVE multiply
                    pr = prp.tile([128, 512], F32, tag="pr", name="pr")
                    nc.tensor.matmul(
                        pr[o_base:o_base + D, :],
                        ones_sel[h % 2][:, 0:D],
                        r,
                        start=True,
                        stop=True,
                    )
                    nc.vector.tensor_mul(
                        oall_sb[h // 2][o_base:o_base + D, ics],
                        ot[o_base:o_base + D, :],
                        pr[o_base:o_base + D, :],
                    )

                pending.append(normalize)

            # ---- schedule ------------------------------------------------
            for h in range(HPG - 1):
                for ic in range(NT_CHUNK):
                    main_block(h, ic, weave_h=h + 1)
            for ic in range(NT_CHUNK):
                main_block(HPG - 1, ic, weave_p3=ic - 1 if ic > 0 else None)
            flush_pending()
            phase3(NT_CHUNK - 1)


def _build_nc(reps=1, debug=False, split_waits=True):
    nc = bass.Bass()
    xT_d = nc.declare_dram_parameter("xT", [E, T], F16, isOutput=False)
    wqkT_d = nc.declare_dram_parameter("wqkT", [E, FQK], F16, isOutput=False)
    wvT_d = nc.declare_dram_parameter("wvT", [E, DV], F16, isOutput=False)
    woutT_d = nc.declare_dram_parameter("woutT", [DV, E], F16, isOutput=False)
    y_d = nc.declare_dram_parameter("y", [T, E], F32, isOutput=True)
    dram = (xT_d, wqkT_d, wvT_d, woutT_d, y_d)
    dbg = None
    if debug:
        shapes = {
            "qk0": [128, T],
            "qk2": [128, T],
            "v0": [128, NT_TILE * DV],
            "qaug0": [D + 1, T],
            "kaug0": [D + 1, T],
            "vaug0": [128, NT_TILE * 128],
            "oall0": [128, T],
            "den0": [NT_CHUNK, 512],
            "sp0": [128, T],
        }
        keys = debug if isinstance(debug, (list, tuple)) else list(shapes)
        dbg = {
            k: nc.declare_dram_parameter(k, shapes[k], F32, isOutput=True)
            for k in keys
        }
    with tile_mod.TileContext(nc) as tc, nc.allow_low_precision(
        reason="fp16 kernel: scores/softmax accumulate in fp32 PSUM; fp16 "
        "elsewhere is validated to rel-err ~5e-3 vs the fp64 reference"
    ):
        for _ in range(reps):
            with tc.tile_pool(name="persist", bufs=1) as persist:
                _emit_body(nc, tc, dram, {"persist": persist}, dbg=dbg)
    if split_waits:
        _split_multi_waits(nc)
    return nc


# ---------------------------------------------------------------------------
# Execution: cached jitted shard_map over 8 cores (axon/PJRT path)
_RUNNERS = {}


class _Runner:
    def __init__(self, reps=1, debug=False):
        import jax
        from jax.sharding import Mesh, PartitionSpec
        from jax.experimental.shard_map import shard_map
        from concourse import bass2jax

        bass2jax.install_neuronx_cc_hook()
        nc = self._nc = _build_nc(reps, debug=debug)

        partition_name = (
            nc.partition_id_tensor.name if nc.partition_id_tensor else None
        )
        in_names, out_names, out_avals, zero_outs = [], [], [], []
        for alloc in nc.m.functions[0].allocations:
            if not isinstance(alloc, mybir.MemoryLocationSet):
                continue
            name = alloc.memorylocations[0].name
            if alloc.kind == "ExternalInput":
                if name != partition_name:
                    in_names.append(name)
            elif alloc.kind == "ExternalOutput":
                shape = tuple(alloc.tensor_shape)
                dtype = mybir.dt.np(alloc.dtype)
                out_names.append(name)
                out_avals.append(jax.core.ShapedArray(shape, dtype))
                zero_outs.append(np.zeros(shape, dtype))
        self.in_names, self.out_names = in_names, out_names
        self.out_avals, self.zero_outs = out_avals, zero_outs
        n_params, n_outs = len(in_names), len(out_names)
        all_in_names = list(in_names) + list(out_names)
        if partition_name is not None:
            all_in_names.append(partition_name)
        all_in_names = tuple(all_in_names)

        def _body(*args):
            operands = list(args)
            if partition_name is not None:
                operands.append(bass2jax.partition_id_tensor())
            outs = bass2jax._bass_exec_p.bind(
                *operands,
                out_avals=tuple(out_avals),
                in_names=all_in_names,
                out_names=tuple(out_names),
                lowering_input_output_aliases=(),
                sim_require_finite=True,
                sim_require_nnan=True,
                nc=nc,
            )
            return tuple(outs)

        devices = jax.devices()[:N_CORES]
        assert len(devices) == N_CORES
        self.mesh = Mesh(np.asarray(devices), ("core",))
        in_specs = (PartitionSpec("core"),) * (n_params + n_outs)
        out_specs = (PartitionSpec("core"),) * n_outs
        self.donate = tuple(range(n_params, n_params + n_outs))
        self.sharded = jax.jit(
            shard_map(
                _body,
                mesh=self.mesh,
                in_specs=in_specs,
                out_specs=out_specs,
                check_rep=False,
            ),
            donate_argnums=self.donate,
            keep_unused=True,
        )

    def stage_inputs(self, per_core_in):
        """per_core_in: list of dicts (len N_CORES) -> device-resident concat arrays."""
        import jax
        from jax.sharding import NamedSharding, PartitionSpec

        sh = NamedSharding(self.mesh, PartitionSpec("core"))
        staged = []
        for name in self.in_names:
            cat = np.concatenate(
                [np.asarray(per_core_in[c][name]) for c in range(N_CORES)], axis=0
            )
            staged.append(jax.device_put(cat, sh))
        return staged

    def fresh_outs(self):
        import jax
        from jax.sharding import NamedSharding, PartitionSpec

        sh = NamedSharding(self.mesh, PartitionSpec("core"))
        return [
            jax.device_put(
                np.zeros((N_CORES * z.shape[0], *z.shape[1:]), z.dtype), sh
            )
            for z in self.zero_outs
        ]

    def run(self, staged_in, out_bufs):
        import jax

        outs = self.sharded(*staged_in, *out_bufs)
        jax.block_until_ready(outs)
        return outs

    def results(self, outs):
        res = []
        for c in range(N_CORES):
            d = {}
            for i, name in enumerate(self.out_names):
                full = np.asarray(outs[i])
                d[name] = full.reshape(N_CORES, *self.out_avals[i].shape)[c]
            res.append(d)
        return res


def _get_runner(reps=1):
    if reps not in _RUNNERS:
        _RUNNERS[reps] = _Runner(reps)
    return _RUNNERS[reps]


# ---------------------------------------------------------------------------
# Host-side sharding / gather
def _per_core_inputs(x, w_qkv, w_out):
    x = np.asarray(x, dtype=np.float32)
    w_qkv = np.asarray(w_qkv, dtype=np.float32)
    w_out = np.asarray(w_out, dtype=np.float32)
    per_core = []
    for c in range(N_CORES):
        b, g = c // GROUPS, c % GROUPS
        hs = np.arange(g * HPG, (g + 1) * HPG)
        # qkv reshape order in reference: f = d*48 + k*16 + h
        rows_q = (np.arange(D)[None, :] * (3 * H_TOTAL) + hs[:, None]).reshape(-1)
        rows_k = rows_q + H_TOTAL
        rows_v = rows_q + 2 * H_TOTAL
        wqk = np.concatenate([w_qkv[rows_q], SCALE * w_qkv[rows_k]], axis=0)
        per_core.append(
            {
                "xT": np.ascontiguousarray(x[b].T).astype(np.float16),
                "wqkT": np.ascontiguousarray(wqk.T).astype(np.float16),
                "wvT": np.ascontiguousarray(w_qkv[rows_v].T).astype(np.float16),
                "woutT": np.ascontiguousarray(w_out[:, g * DV:(g + 1) * DV].T).astype(np.float16),
            }
        )
    return per_core


def kernel(x, w_qkv, w_out):
    runner = _get_runner(1)
    staged = runner.stage_inputs(_per_core_inputs(x, w_qkv, w_out))
    outs = runner.run(staged, runner.fresh_outs())
    res = runner.results(outs)
    y = np.zeros((B, T, E), dtype=np.float64)
    for c in range(N_CORES):
        y[c // GROUPS] += res[c]["y"].astype(np.float64)
    return y.astype(np.float32)



# revision 17
# speedup vs baseline: 2.0639x; 1.0023x over previous
"""Trainium2 Bass kernel for 16-head MHA (B=2, T=2048, E=1024), SPMD on 8 cores.

Sharding: data-parallel over batch (2) x tensor-parallel over heads (4 groups
of 4 heads). Each core computes, for its (batch b, head-group g):
  qk^T projection (feature-major), v projection (token-major),
  shifted-softmax attention via an augmented-row matmul trick, and a partial
  output projection over its 256 embedding columns. The host sums the 4
  partial projections per batch.

Softmax shift: the exact per-query max over all keys is computed on-device
(q-stationary matmul in [i, j] orientation + free-dim DVE reduces) and folded
into the main QK^T matmul as a rank-1 augmented row, so scores arrive in PSUM
already shifted: S'[j,i] = 8*q_i.k_j - M_i. exp() runs on ACT straight out of
PSUM. The softmax denominator comes for free from a ones-column appended to V.
"""

import sys

sys.path.insert(0, "/opt/trn_rl_repo")

import numpy as np

import concourse.bass as bass
import concourse.mybir as mybir
import concourse.tile as tile_mod
from concourse.masks import make_identity

F32 = mybir.dt.float32
F16 = mybir.dt.float16

B, T, E = 2, 2048, 1024
H_TOTAL, D = 16, 64
N_CORES = 8
GROUPS = 4          # head-group (tensor) parallelism
HPG = H_TOTAL // GROUPS  # 4 heads per group
DV = HPG * D        # 256: v width / out-proj contraction per core
FQK = 2 * DV        # 512: q+k feature rows per core
SCALE = float(np.sqrt(D))  # reference MULTIPLIES scores by sqrt(d)

NE = E // 128       # 8 e-chunks
NT_TILE = T // 128  # 16 token tiles
NT_CHUNK = T // 512  # 4 token chunks
N_SAMPLE_TILES = 2  # 256-key sample for the softmax shift


# ---------------------------------------------------------------------------
# Workaround: this walrus build only accepts ONE sem wait per instruction.
# After Tile scheduling, split every multi-wait instruction: the overflow
# waits move onto same-engine NoOps inserted immediately before it.
def _split_multi_waits(nc):
    for f in nc.m.functions:
        for bb in f.blocks:
            out = []
            for inst in bb.instructions:
                si = getattr(inst, "sync_info", None)
                if si is not None and si.on_wait and len(si.on_wait) > 1:
                    extras = list(si.on_wait[:-1])
                    si.on_wait = list(si.on_wait[-1:])
                    for w in extras:
                        nop = mybir.InstNoOp(
                            name=f"I-{nc.next_id()}", ins=[], outs=[]
                        )
                        nop.engine = inst.engine
                        nop.sync_info = mybir.SyncInfo(on_wait=[w], on_update=[])
                        out.append(nop)
                out.append(inst)
            bb.instructions[:] = out


# ---------------------------------------------------------------------------
# Device program (identical on every core; per-core data differs)
#
# All matmul operands are fp16 and K-padded to the full 128 partitions
# (zero rows) -- K=64/65 matmuls stream rhs columns at half rate on TRN2
# (measured 513ns vs 265ns for N=512), so padding doubles PE throughput.
# Scheduling: head h+1's max pass (PE matmuls + DVE reduces) weaves through
# head h's QK/exp/PV blocks; head 0's max pass weaves through the phase-1
# projections; the out-projection for chunk ic weaves through head 3's
# block ic+1. The per-head softmax denominator reciprocal runs on ACT as
# exp(-ln(d)) so the slow single-partition DVE reciprocal never gates the
# PE's broadcast matmul.
def _emit_body(nc, tc, dram, ctx_pools, dbg=None):
    xT_d, wqkT_d, wvT_d, woutT_d, y_d = dram
    persist = ctx_pools["persist"]

    # persistent SBUF
    qk_sb = [persist.tile([128, T], F16, tag=f"qk{i}", name=f"qk{i}") for i in range(FQK // 128)]
    # v as one [128, 16*256] tile: [t-tile partition, (jt, dv)] layout
    v_sb = persist.tile([128, NT_TILE * DV], F16, tag="v", name="v")
    oall_sb = [persist.tile([128, T], F16, tag=f"oall{i}", name=f"oall{i}") for i in range(DV // 128)]
    wout_sb = [persist.tile([128, E], F16, tag=f"wout{i}", name=f"wout{i}") for i in range(DV // 128)]
    # one-hot row selectors for the reciprocal broadcast (K=128 matmul):
    # ones_sel[par][den] row = 1, all else 0; junk rows of r are multiplied
    # by zero so only the denominator row is broadcast.
    ones_sel = []
    for par, dn in ((0, D), (1, 0)):
        t = persist.tile([128, D], F16, tag=f"ones{par}", name=f"ones{par}")
        nc.vector.memset(t, 0.0)
        nc.vector.memset(t[dn:dn + 1, :], 1.0)
        ones_sel.append(t)
    identity = persist.tile([128, 128], F32, tag="identity", name="identity")
    make_identity(nc, identity)
    for i in range(DV // 128):
        nc.sync.dma_start(out=wout_sb[i], in_=woutT_d[i * 128:(i + 1) * 128, :])

    with (
        tc.tile_pool(name="aug", bufs=1) as augp,
        tc.tile_pool(name="vaug", bufs=1) as vaugp,
        tc.tile_pool(name="mx", bufs=1) as mxp,
        tc.tile_pool(name="px", bufs=2, space="PSUM") as pxp,
        tc.tile_pool(name="mx4", bufs=1, space="PSUM") as mx4p,
    ):
        k_augs, q_augs, vaugs = [], [], []

        def build_qk_aug(h):
            # [128, T]: rows 0:64 = q|k, row 64 = -M slot (q, zero-init) /
            # ones (k), rows 65:128 = zero K-padding. Odd heads' rows live
            # at partitions 64..127 of qk_sb -> partition-shifting DMA.
            odd = h % 2 == 1
            q_tile, k_tile = h // 2, 2 + h // 2
            k_aug = augp.tile([128, T], F16, tag=f"kaug{h}", name=f"kaug{h}")
            q_aug = augp.tile([128, T], F16, tag=f"qaug{h}", name=f"qaug{h}")
            if odd:
                nc.sync.dma_start(out=k_aug[0:D, :], in_=qk_sb[k_tile][D:2 * D, :])
                nc.sync.dma_start(out=q_aug[0:D, :], in_=qk_sb[q_tile][D:2 * D, :])
            else:
                nc.vector.tensor_copy(out=k_aug[0:D, :], in_=qk_sb[k_tile][0:D, :])
                nc.vector.tensor_copy(out=q_aug[0:D, :], in_=qk_sb[q_tile][0:D, :])
            nc.vector.memset(q_aug[D:128, :], 0.0)
            nc.vector.memset(k_aug[D:128, :], 0.0)
            nc.vector.memset(k_aug[D:D + 1, :], 1.0)
            k_augs.append(k_aug)
            q_augs.append(q_aug)

        # ---- max-pass emitters ----------------------------------------
        # scores in [i-part, j-free] orientation (q-stationary, K=128
        # zero-padded); row max = DVE free-axis reduce; PE transpose lays
        # -M along the free dim of q_aug row D via one small DMA.
        max_state = {}

        def max_quad(h, ic, c):
            q_aug, k_aug = q_augs[h], k_augs[h]
            if c == 0:
                max_state[(h, ic)] = (
                    mxp.tile([128, NT_CHUNK], F32, tag="mxt", name="mxt", bufs=2),
                    mxp.tile([128, NT_CHUNK * 3], F32, tag="mxt2", name="mxt2", bufs=2),
                )
            mxt, mxt2 = max_state[(h, ic)]
            m2r = mxt2.rearrange("p (c j) -> p c j", j=3)
            it = ic * NT_CHUNK + c
            for jc in range(NT_CHUNK):
                s_ps = pxp.tile([128, 512], F32, tag="px", name="px")
                nc.tensor.matmul(
                    s_ps,
                    q_aug[:, it * 128:(it + 1) * 128],
                    k_aug[:, jc * 512:(jc + 1) * 512],
                    start=True,
                    stop=True,
                )
                if jc == 0:
                    nc.vector.reduce_max(
                        out=mxt[:, c:c + 1], in_=s_ps,
                        axis=mybir.AxisListType.X,
                    )
                else:
                    nc.vector.reduce_max(
                        out=m2r[:, c, jc - 1:jc], in_=s_ps,
                        axis=mybir.AxisListType.X,
                    )

        def max_finish(h, ic):
            q_aug = q_augs[h]
            ics = slice(ic * 512, (ic + 1) * 512)
            mxt, mxt2 = max_state.pop((h, ic))
            m2r = mxt2.rearrange("p (c j) -> p c j", j=3)
            mc = mxp.tile([128, NT_CHUNK], F32, tag="mc", name="mc", bufs=2)
            nc.vector.tensor_max(mc, m2r[:, :, 0], m2r[:, :, 1])
            nc.vector.tensor_max(mc, mc, m2r[:, :, 2])
            nc.vector.tensor_max(mxt, mxt, mc)
            mx4 = mx4p.tile([NT_CHUNK, 128], F32, tag="mx4", name="mx4")
            nc.tensor.transpose(mx4, mxt, identity)
            mrow = mxp.tile([NT_CHUNK, 128], F16, tag="mrow", name="mrow", bufs=2)
            nc.vector.tensor_scalar_mul(mrow, mx4, -1.0)
            nc.sync.dma_start(
                out=q_aug[D:D + 1, ics].rearrange(
                    "p (c f) -> p c f", c=NT_CHUNK
                ),
                in_=mrow,
            )

        def max_chunk(h, ic):
            for c in range(NT_CHUNK):
                max_quad(h, ic, c)
            max_finish(h, ic)

        # ---- Phase 1: projections, with head 0's max pass woven in ----
        # ff order (0,2,1,3): heads 0/1 need qk tiles 0 and 2 first.
        # PSUM->SBUF copies go via ACT so the DVE is free for aug builds
        # and max reduces.
        with (
            tc.tile_pool(name="ph1", bufs=1) as ph1,
            tc.tile_pool(name="pj", bufs=4, space="PSUM") as pj,
            tc.tile_pool(name="pv", bufs=1, space="PSUM") as pv,
        ):
            xt_sb = [ph1.tile([128, T], F16, tag=f"xt{i}", name=f"xt{i}") for i in range(NE)]
            wqk_sb = [ph1.tile([128, FQK], F16, tag=f"wqk{i}", name=f"wqk{i}") for i in range(NE)]
            wv_sb = [ph1.tile([128, DV], F16, tag=f"wv{i}", name=f"wv{i}") for i in range(NE)]
            # two DMA queues: weights on SP, x/wv on the ACT queue, so the
            # first projection's operands arrive in parallel.
            for i in range(NE):
                nc.sync.dma_start(out=wqk_sb[i], in_=wqkT_d[i * 128:(i + 1) * 128, :])
                nc.scalar.dma_start(out=xt_sb[i], in_=xT_d[i * 128:(i + 1) * 128, :])
                nc.scalar.dma_start(out=wv_sb[i], in_=wvT_d[i * 128:(i + 1) * 128, :])

            def qk_proj(ff):
                # qk^T [f', t] = W'[f', e] @ x^T[e, t], feature-major
                ps = [pj.tile([128, 512], F32, tag="pj", name="pj") for _ in range(NT_CHUNK)]
                for ne in range(NE):
                    lhsT = wqk_sb[ne][:, ff * 128:(ff + 1) * 128]
                    for tt in range(NT_CHUNK):
                        nc.tensor.matmul(
                            ps[tt],
                            lhsT,
                            xt_sb[ne][:, tt * 512:(tt + 1) * 512],
                            start=(ne == 0),
                            stop=(ne == NE - 1),
                        )
                for tt in range(NT_CHUNK):
                    nc.scalar.activation(
                        out=qk_sb[ff][:, tt * 512:(tt + 1) * 512], in_=ps[tt],
                        func=mybir.ActivationFunctionType.Copy,
                    )

            qk_proj(0)
            qk_proj(2)
            build_qk_aug(0)
            build_qk_aug(1)
            qk_proj(1)
            max_chunk(0, 0)
            qk_proj(3)
            build_qk_aug(2)
            build_qk_aug(3)
            max_chunk(0, 1)

            # v [t, dv] token-major, two tiles per full-bank PSUM buffer
            # (half-bank tiles measured pathologically slow); head 0's
            # remaining max chunks weave between pairs.
            for tp in range(NT_TILE // 2):
                psv = pv.tile([128, 512], F32, tag="pv", name="pv")
                for half in range(2):
                    tj = tp * 2 + half
                    for ne in range(NE):
                        nc.tensor.matmul(
                            psv[:, half * DV:(half + 1) * DV],
                            xt_sb[ne][:, tj * 128:(tj + 1) * 128],
                            wv_sb[ne],
                            start=(ne == 0),
                            stop=(ne == NE - 1),
                        )
                nc.scalar.activation(
                    out=v_sb[:, tp * 2 * DV:(tp + 1) * 2 * DV], in_=psv,
                    func=mybir.ActivationFunctionType.Copy,
                )
                if tp == 4:
                    max_chunk(0, 2)
                elif tp == 6:
                    max_chunk(0, 3)

        for h in range(HPG):
            odd = h % 2 == 1
            # V-stationary [128, 16*128]: per j-tile 128 columns holding
            # (even) [v(64) | ones | 0...]   -> O rows 0..63, denom row 64
            # (odd)  [ones | 0... | v(64)]   -> denom row 0, O rows 64..127
            vaug = vaugp.tile([128, NT_TILE * 128], F16, tag=f"vaug{h}", name=f"vaug{h}")
            va_r = vaug.rearrange("p (j c) -> p j c", c=128)
            v_r = v_sb.rearrange("p (j c) -> p j c", c=DV)
            eng = nc.gpsimd if odd else nc.vector
            if odd:
                eng.memset(va_r[:, :, 0:1], 1.0)
                eng.memset(va_r[:, :, 1:D], 0.0)
                eng.tensor_copy(out=va_r[:, :, D:2 * D], in_=v_r[:, :, h * D:(h + 1) * D])
            else:
                eng.tensor_copy(out=va_r[:, :, 0:D], in_=v_r[:, :, h * D:(h + 1) * D])
                eng.memset(va_r[:, :, D:D + 1], 1.0)
                eng.memset(va_r[:, :, D + 1:128], 0.0)
            vaugs.append(vaug)

        # ---- Phase 2 main + phase 3 ------------------------------------
        with (
            tc.tile_pool(name="pt", bufs=4) as ptp,
            tc.tile_pool(name="rr", bufs=1) as rrp,
            tc.tile_pool(name="ot", bufs=2) as otp,
            tc.tile_pool(name="ysb", bufs=3) as ysbp,
            tc.tile_pool(name="ps", bufs=2, space="PSUM") as psp,
            tc.tile_pool(name="po", bufs=2, space="PSUM") as pop,
            tc.tile_pool(name="pr", bufs=1, space="PSUM") as prp,
        ):
            # per-head persistent r tiles, zeroed once: the broadcast
            # matmul streams all 128 rows of r, so the 0-weighted rows must
            # hold finite values; only the denominator row is ever rewritten.
            r_tiles = []
            for h in range(HPG):
                rt = rrp.tile([128, 512], F16, tag=f"r{h}", name=f"r{h}", bufs=1)
                nc.vector.memset(rt, 0.0)
                r_tiles.append(rt)

            pending = []  # delayed (PE broadcast + DVE mul) closures

            def flush_pending():
                while pending:
                    pending.pop(0)()

            def phase3(ic):
                # out-projection for a finished token chunk; PSUM comes
                # from the (idle by now) max-pass pool pxp.
                for tt in range(ic * NT_CHUNK, (ic + 1) * NT_CHUNK):
                    pys = [pxp.tile([128, 512], F32, tag="px", name="px") for _ in range(2)]
                    for es in range(DV // 128):
                        lhsT = oall_sb[es][:, tt * 128:(tt + 1) * 128]
                        for oc in range(2):
                            nc.tensor.matmul(
                                pys[oc],
                                lhsT,
                                wout_sb[es][:, oc * 512:(oc + 1) * 512],
                                start=(es == 0),
                                stop=(es == DV // 128 - 1),
                            )
                    yt = ysbp.tile([128, E], F32, tag="y", name="y")
                    for oc in range(2):
                        nc.vector.tensor_copy(out=yt[:, oc * 512:(oc + 1) * 512], in_=pys[oc])
                    nc.sync.dma_start(out=y_d[tt * 128:(tt + 1) * 128, :], in_=yt)

            def main_block(h, ic, weave_h=None, weave_p3=None):
                odd = h % 2 == 1
                o_base = (h % 2) * D
                den = D if not odd else 0
                ics = slice(ic * 512, (ic + 1) * 512)
                q_aug, k_aug, vaug = q_augs[h], k_augs[h], vaugs[h]
                po = pop.tile([128, 512], F32, tag="po", name="po")
                for jq in range(NT_CHUNK):
                    if weave_h is not None:
                        max_quad(weave_h, ic, jq)
                    for jt in range(jq * NT_CHUNK, (jq + 1) * NT_CHUNK):
                        ps = psp.tile([128, 512], F32, tag="ps", name="ps")
                        nc.tensor.matmul(
                            ps,
                            k_aug[:, jt * 128:(jt + 1) * 128],
                            q_aug[:, ics],
                            start=True,
                            stop=True,
                        )
                        pT = ptp.tile([128, 512], F16, tag="pt", name="pt")
                        nc.scalar.activation(
                            out=pT, in_=ps, func=mybir.ActivationFunctionType.Exp
                        )
                        nc.tensor.matmul(
                            po,
                            vaug[:, jt * 128:(jt + 1) * 128],
                            pT,
                            start=(jt == 0),
                            stop=(jt == NT_TILE - 1),
                        )
                if weave_h is not None:
                    max_finish(weave_h, ic)
                flush_pending()
                if weave_p3 is not None:
                    phase3(weave_p3)
                # eager: free po's bank fast. Reciprocal on ACT as
                # exp(-ln(d)) -- the exact DVE reciprocal is a serial
                # 3.35us op that gated the broadcast matmul.
                r = r_tiles[h]
                lnt = otp.tile([128, 512], F32, tag="lnt", name="lnt", bufs=2)
                nc.scalar.activation(
                    out=lnt[den:den + 1, :], in_=po[den:den + 1, :],
                    func=mybir.ActivationFunctionType.Ln,
                )
                nc.scalar.activation(
                    out=r[den:den + 1, :], in_=lnt[den:den + 1, :],
                    func=mybir.ActivationFunctionType.Exp, scale=-1.0,
                )
                ot = otp.tile([128, 512], F32, tag="ot", name="ot")
                nc.vector.tensor_copy(
                    out=ot[o_base:o_base + D, :], in_=po[o_base:o_base + D, :]
                )

                def normalize():
                    # delayed: PE broadcast of r across O rows + DVE multiply
                    pr = prp.tile([128, 512], F32, tag="pr", name="pr")
                    nc.tensor.matmul(
                        pr[o_base:o_base + D, :],
                        ones_sel[h % 2][:, 0:D],
                        r,
                        start=True,
                        stop=True,
                    )
                    nc.vector.tensor_mul(
                        oall_sb[h // 2][o_base:o_base + D, ics],
                        ot[o_base:o_base + D, :],
                        pr[o_base:o_base + D, :],
                    )

                pending.append(normalize)

            # ---- schedule ------------------------------------------------
            for h in range(HPG - 1):
                for ic in range(NT_CHUNK):
                    main_block(h, ic, weave_h=h + 1)
            for ic in range(NT_CHUNK):
                main_block(HPG - 1, ic, weave_p3=ic - 1 if ic > 0 else None)
            flush_pending()
            phase3(NT_CHUNK - 1)


def _build_nc(reps=1, debug=False, split_waits=True):
    nc = bass.Bass()
    xT_d = nc.declare_dram_parameter("xT", [E, T], F16, isOutput=False)
    wqkT_d = nc.declare_dram_parameter("wqkT", [E, FQK], F16, isOutput=False)
    wvT_d = nc.declare_dram_parameter("wvT", [E, DV], F16, isOutput=False)
    woutT_d = nc.declare_dram_parameter("woutT", [DV, E], F16, isOutput=False)
    y_d = nc.declare_dram_parameter("y", [T, E], F32, isOutput=True)
    dram = (xT_d, wqkT_d, wvT_d, woutT_d, y_d)
    dbg = None
    if debug:
        shapes = {
            "qk0": [128, T],
            "qk2": [128, T],
            "v0": [128, NT_TILE * DV],
            "qaug0": [D + 1, T],
            "kaug0": [D + 1, T],
            "vaug0": [128, NT_TILE * 128],
            "oall0": [128, T],
            "den0": [NT_CHUNK, 512],
            "sp0": [128, T],
        }
        keys = debug if isinstance(debug, (list, tuple)) else list(shapes)
        dbg = {
            k: nc.declare_dram_parameter(k, shapes[k], F32, isOutput=True)
            for k in keys
        }
    with tile_mod.TileContext(nc) as tc, nc.allow_low_precision(
        reason="fp16 kernel: scores/softmax accumulate in fp32 PSUM; fp16 "
        "elsewhere is validated to rel-err ~5e-3 vs the fp64 reference"
    ):
        for _ in range(reps):
            with tc.tile_pool(name="persist", bufs=1) as persist:
                _emit_body(nc, tc, dram, {"persist": persist}, dbg=dbg)
    if split_waits:
        _split_multi_waits(nc)
    return nc


# ---------------------------------------------------------------------------
# Execution: cached jitted shard_map over 8 cores (axon/PJRT path)
_RUNNERS = {}


class _Runner:
    def __init__(self, reps=1, debug=False):
        import jax
        from jax.sharding import Mesh, PartitionSpec
        from jax.experimental.shard_map import shard_map
        from concourse import bass2jax

        bass2jax.install_neuronx_cc_hook()
        nc = self._nc = _build_nc(reps, debug=debug)

        partition_name = (
            nc.partition_id_tensor.name if nc.partition_id_tensor else None
        )
        in_names, out_names, out_avals, zero_outs = [], [], [], []
        for alloc in nc.m.functions[0].allocations:
            if not isinstance(alloc, mybir.MemoryLocationSet):
                continue
            name = alloc.memorylocations[0].name
            if alloc.kind == "ExternalInput":
                if name != partition_name:
                    in_names.append(name)
            elif alloc.kind == "ExternalOutput":
                shape = tuple(alloc.tensor_shape)
                dtype = mybir.dt.np(alloc.dtype)
                out_names.append(name)
                out_avals.append(jax.core.ShapedArray(shape, dtype))
                zero_outs.append(np.zeros(shape, dtype))
        self.in_names, self.out_names = in_names, out_names
        self.out_avals, self.zero_outs = out_avals, zero_outs
        n_params, n_outs = len(in_names), len(out_names)
        all_in_names = list(in_names) + list(out_names)
        if partition_name is not None:
            all_in_names.append(partition_name)
        all_in_names = tuple(all_in_names)

        def _body(*args):
            operands = list(args)
            if partition_name is not None:
                operands.append(bass2jax.partition_id_tensor())
            outs = bass2jax._bass_exec_p.bind(
                *operands,
                out_avals=tuple(out_avals),
                in_names=all_in_names,
                out_names=tuple(out_names),
                lowering_input_output_aliases=(),
                sim_require_finite=True,
                sim_require_nnan=True,
                nc=nc,
            )
            return tuple(outs)

        devices = jax.devices()[:N_CORES]
        assert len(devices) == N_CORES
        self.mesh = Mesh(np.asarray(devices), ("core",))
        in_specs = (PartitionSpec("core"),) * (n_params + n_outs)
        out_specs = (PartitionSpec("core"),) * n_outs
        self.donate = tuple(range(n_params, n_params + n_outs))
        self.sharded = jax.jit(
            shard_map(
                _body,
                mesh=self.mesh,
                in_specs=in_specs,
                out_specs=out_specs,
                check_rep=False,
            ),
            donate_argnums=self.donate,
            keep_unused=True,
        )

    def stage_inputs(self, per_core_in):
        """per_core_in: list of dicts (len N_CORES) -> device-resident concat arrays."""
        import jax
        from jax.sharding import NamedSharding, PartitionSpec

        sh = NamedSharding(self.mesh, PartitionSpec("core"))
        staged = []
        for name in self.in_names:
            cat = np.concatenate(
                [np.asarray(per_core_in[c][name]) for c in range(N_CORES)], axis=0
            )
            staged.append(jax.device_put(cat, sh))
        return staged

    def fresh_outs(self):
        import jax
        from jax.sharding import NamedSharding, PartitionSpec

        sh = NamedSharding(self.mesh, PartitionSpec("core"))
        return [
            jax.device_put(
                np.zeros((N_CORES * z.shape[0], *z.shape[1:]), z.dtype), sh
            )
            for z in self.zero_outs
        ]

    def run(self, staged_in, out_bufs):
        import jax

        outs = self.sharded(*staged_in, *out_bufs)
        jax.block_until_ready(outs)
        return outs

    def results(self, outs):
        res = []
        for c in range(N_CORES):
            d = {}
            for i, name in enumerate(self.out_names):
                full = np.asarray(outs[i])
                d[name] = full.reshape(N_CORES, *self.out_avals[i].shape)[c]
            res.append(d)
        return res


def _get_runner(reps=1):
    if reps not in _RUNNERS:
        _RUNNERS[reps] = _Runner(reps)
    return _RUNNERS[reps]


# ---------------------------------------------------------------------------
# Host-side sharding / gather
def _per_core_inputs(x, w_qkv, w_out):
    x = np.asarray(x, dtype=np.float32)
    w_qkv = np.asarray(w_qkv, dtype=np.float32)
    w_out = np.asarray(w_out, dtype=np.float32)
    per_core = []
    for c in range(N_CORES):
        b, g = c // GROUPS, c % GROUPS
        hs = np.arange(g * HPG, (g + 1) * HPG)
        # qkv reshape order in reference: f = d*48 + k*16 + h
        rows_q = (np.arange(D)[None, :] * (3 * H_TOTAL) + hs[:, None]).reshape(-1)
        rows_k = rows_q + H_TOTAL
        rows_v = rows_q + 2 * H_TOTAL
        wqk = np.concatenate([w_qkv[rows_q], SCALE * w_qkv[rows_k]], axis=0)
        per_core.append(
            {
                "xT": np.ascontiguousarray(x[b].T).astype(np.float16),
                "wqkT": np.ascontiguousarray(wqk.T).astype(np.float16),
                "wvT": np.ascontiguousarray(w_qkv[rows_v].T).astype(np.float16),
                "woutT": np.ascontiguousarray(w_out[:, g * DV:(g + 1) * DV].T).astype(np.float16),
            }
        )
    return per_core


def kernel(x, w_qkv, w_out):
    runner = _get_runner(1)
    staged = runner.stage_inputs(_per_core_inputs(x, w_qkv, w_out))
    outs = runner.run(staged, runner.fresh_outs())
    res = runner.results(outs)
    y = np.zeros((B, T, E), dtype=np.float64)
    for c in range(N_CORES):
        y[c // GROUPS] += res[c]["y"].astype(np.float64)
    return y.astype(np.float32)

